# revision 26
# baseline (speedup 1.0000x reference)
"""Trainium2 Bass kernel for the ActiveParticles nn.Module (N=4096, 8 cores).

Strategy
--------
Host sorts particles by post-translation x coordinate (a pure permutation —
the reference computation is permutation-equivariant). In sorted space every
interaction radius becomes a *rank band*:
  - collision radius 2*RC  -> band +-128 ranks   (30-iteration loop)
  - orientation radius RO+RC -> band +-384 ranks (one-time masks)
Each of the 8 cores owns 512 consecutive sorted rows and computes only a
[128 x band] tile strip per row-tile. The collision loop allgathers the
updated 512x2 positions every iteration. Band validity is *proved* per input:
host-side exact gap checks + a device-computed max-displacement guard; if any
check fails, a dense numpy fallback (exact reference semantics) is used.

Outputs are produced in sorted space and unpermuted on host.
"""

import os
import numpy as np

# ---------------------------------------------------------------- constants
PI = 3.1415927410125732
N = 4096
L = 1.28e-3
VEL = 5e-7
RC = 3.15e-6
RR = 8e-6
RO = 2.5e-5
DT_TRANS = 1.4e-14
DR = 0.0028
GAMMA = 25.0
DT = 0.2
COLL_ITERS = 30

NCORES = 8
ROWS = N // NCORES          # 512 rows per core
NT = ROWS // 128            # 4 row-tiles per core

BC = 128                    # collision band (ranks each side)
CW = 2 * BC + 128           # collision tile window = 384
CWIN = ROWS + 2 * BC        # per-core collision window = 768

BF = 384                    # forward (mask) band
FW = 2 * BF + 128           # forward tile window = 896
FWIN = ROWS + 2 * BF        # per-core forward window = 1280

PAD = 128                   # sentinel pad rows each side of the shared array
NPAD = N + 2 * PAD          # 4352
SENT = np.float32(1.0e9)

F32 = np.float32
C2 = F32((2.0 * RC) ** 2)          # collision threshold on squared dist
FAC = F32(1.05 * RC)               # (2.1*RC)*0.5
RR2 = F32(RR ** 2)
RO2 = F32((RO + RC) ** 2)
C_DV = F32(DT * VEL)
C_TN = F32(np.sqrt(0.5) * np.sqrt(2.0 * DT_TRANS) * np.sqrt(DT))
C_GGD = F32(DT * GAMMA * DR)
C_RN = F32(np.sqrt(2.0 * DR) * np.sqrt(DT))
HALF_PI = F32(PI / 2)

_BUILT = None
TRACE = False               # test.py sets True to collect exec_time_ns
LAST_EXEC_NS = None


# =====================================================================
# Device program
# =====================================================================
def _build_nc():
    import concourse.bass as bass
    import concourse.mybir as mybir
    from concourse import tile, bass_isa, library_config
    from concourse import bacc

    f32 = mybir.dt.float32
    A = mybir.AluOpType
    AF = mybir.ActivationFunctionType

    nc = bacc.Bacc("TRN2", target_bir_lowering=False, debug=False,
                   num_devices=NCORES)

    # ---------------- I/O ----------------
    t_own = nc.dram_tensor("own8", [128, NT, 8], f32, kind="ExternalInput")
    t_cwin = nc.dram_tensor("cwin", [6, CWIN], f32, kind="ExternalInput")
    t_fwin = nc.dram_tensor("fwin", [4, FWIN], f32, kind="ExternalInput")
    t_fpx = nc.dram_tensor("fpx", [1, N], f32, kind="ExternalInput")
    t_fpy = nc.dram_tensor("fpy", [1, N], f32, kind="ExternalInput")
    t_eye = nc.dram_tensor("eye128", [128, 128], f32, kind="ExternalInput")

    o_pos = nc.dram_tensor("o_pos", [128, NT, 2], f32, kind="ExternalOutput")
    o_ori = nc.dram_tensor("o_ori", [128, NT, 2], f32, kind="ExternalOutput")
    o_osum = nc.dram_tensor("o_osum", [128, NT, 2], f32, kind="ExternalOutput")
    o_left = nc.dram_tensor("o_left", [128, NT, 2], f32, kind="ExternalOutput")
    o_right = nc.dram_tensor("o_right", [128, NT, 2], f32, kind="ExternalOutput")
    o_guard = nc.dram_tensor("o_guard", [128, NT, 2], f32, kind="ExternalOutput")

    rg = [list(range(NCORES))]

    with tile.TileContext(nc) as tc:
        dve, act, gps, dma = nc.vector, nc.scalar, nc.gpsimd, nc.sync
        with tc.tile_pool(name="main", bufs=1) as mp, \
             tc.tile_pool(name="dram", bufs=2, space="DRAM") as dp:

            def T(shape, name, dt=f32):
                return mp.tile(shape, dt, name=name, tag=name, bufs=1)

            # persistent SBUF state
            own = T([128, NT, 8], "own")
            cwx = T([1, CWIN], "cwx")
            cwy = T([1, CWIN], "cwy")
            cwo1 = T([1, CWIN], "cwo1")
            cwo2 = T([1, CWIN], "cwo2")
            cwt1 = T([1, CWIN], "cwt1")
            cwt2 = T([1, CWIN], "cwt2")
            fwx = T([1, FWIN], "fwx")
            fwy = T([1, FWIN], "fwy")
            fwo1 = T([1, FWIN], "fwo1")
            fwo2 = T([1, FWIN], "fwo2")
            eye = T([128, 128], "eye")
            fpx = T([1, N], "fpx")
            fpy = T([1, N], "fpy")
            px = T([128, NT], "px")
            py = T([128, NT], "py")
            px0 = T([128, NT], "px0")
            py0 = T([128, NT], "py0")
            gx = T([128, NT], "gx")
            gy = T([128, NT], "gy")
            sumx = T([128, NT], "sumx")
            sumy = T([128, NT], "sumy")
            nrt_ = T([128, NT], "nrt")
            sxt = T([128, NT], "sxt")
            syt = T([128, NT], "syt")
            oxt = T([128, NT], "oxt")
            oyt = T([128, NT], "oyt")
            cbias = T([128, 1], "cbias")

            dma.dma_start(own[:], t_own[:])
            dma.dma_start(cwx[:], t_cwin[0:1, :])
            dma.dma_start(cwy[:], t_cwin[1:2, :])
            dma.dma_start(cwo1[:], t_cwin[2:3, :])
            dma.dma_start(cwo2[:], t_cwin[3:4, :])
            dma.dma_start(cwt1[:], t_cwin[4:5, :])
            dma.dma_start(cwt2[:], t_cwin[5:6, :])
            dma.dma_start(fwx[:], t_fwin[0:1, :])
            dma.dma_start(fwy[:], t_fwin[1:2, :])
            dma.dma_start(fwo1[:], t_fwin[2:3, :])
            dma.dma_start(fwo2[:], t_fwin[3:4, :])
            dma.dma_start(eye[:], t_eye[:])
            dma.dma_start(fpx[:], t_fpx[:])
            dma.dma_start(fpy[:], t_fpy[:])
            dve.memset(cbias[:], float(HALF_PI))


            def ow(k):
                return own[:, :, k]

            # seed p0 = pos + trans (own rows)
            dve.scalar_tensor_tensor(px[:], ow(2), float(C_DV), ow(0), A.mult, A.add)
            dve.scalar_tensor_tensor(px[:], ow(6), float(C_TN), px[:], A.mult, A.add)
            dve.scalar_tensor_tensor(py[:], ow(3), float(C_DV), ow(1), A.mult, A.add)
            dve.scalar_tensor_tensor(py[:], ow(7), float(C_TN), py[:], A.mult, A.add)
            dve.tensor_copy(px0[:], px[:])
            dve.tensor_copy(py0[:], py[:])
            dve.memset(gx[:], 0.0)
            dve.memset(gy[:], 0.0)

            # seed window p0 = pos + trans
            dve.scalar_tensor_tensor(cwx[:], cwo1[:], float(C_DV), cwx[:],
                                     A.mult, A.add)
            dve.scalar_tensor_tensor(cwx[:], cwt1[:], float(C_TN), cwx[:],
                                     A.mult, A.add)
            dve.scalar_tensor_tensor(cwy[:], cwo2[:], float(C_DV), cwy[:],
                                     A.mult, A.add)
            dve.scalar_tensor_tensor(cwy[:], cwt2[:], float(C_TN), cwy[:],
                                     A.mult, A.add)

            # =====================================================
            # FORWARD one-time: masks + reductions (band BF)
            # =====================================================
            with tc.tile_pool(name="fsb", bufs=2) as fp:
                pxb = fp.tile([128, FWIN], f32, name="pxb", tag="pxb", bufs=1)
                pyb = fp.tile([128, FWIN], f32, name="pyb", tag="pyb", bufs=1)
                oreb = fp.tile([128, FWIN], f32, name="oreb", tag="oreb", bufs=1)
                oimb = fp.tile([128, FWIN], f32, name="oimb", tag="oimb", bufs=1)
                gps.partition_broadcast(pxb[:], fwx[:])
                gps.partition_broadcast(pyb[:], fwy[:])
                gps.partition_broadcast(oreb[:], fwo1[:])
                gps.partition_broadcast(oimb[:], fwo2[:])

                for t in range(NT):
                    w0 = 128 * t
                    pxi = own[:, t, 0:1]
                    pyi = own[:, t, 1:2]
                    orei = own[:, t, 2:3]
                    oimi = own[:, t, 3:4]
                    fdx = fp.tile([128, FW], f32, tag="fdx")
                    fdy = fp.tile([128, FW], f32, tag="fdy")
                    ft1 = fp.tile([128, FW], f32, tag="ft1")
                    ft2 = fp.tile([128, FW], f32, tag="ft2")
                    fsq = fp.tile([128, FW], f32, tag="fsq")
                    fg = fp.tile([128, FW], f32, tag="fg")
                    fm1 = fp.tile([128, FW], f32, tag="fm1")
                    fm2 = fp.tile([128, FW], f32, tag="fm2")
                    fwrr = fp.tile([128, FW], f32, tag="fwrr")
                    fwro = fp.tile([128, FW], f32, tag="fwro")

                    dve.tensor_scalar(fdx[:], pxb[:, w0:w0 + FW], pxi, None,
                                      A.subtract)
                    dve.tensor_scalar(fdy[:], pyb[:, w0:w0 + FW], pyi, None,
                                      A.subtract)
                    dve.tensor_tensor(fdx[:, BF:BF + 128], fdx[:, BF:BF + 128],
                                      eye[:], A.add)
                    act.activation(ft1[:], fdx[:], AF.Square)
                    act.activation(ft2[:], fdy[:], AF.Square)
                    dve.tensor_tensor(fsq[:], ft1[:], ft2[:], A.add)
                    dve.tensor_scalar(fg[:], oimb[:, w0:w0 + FW], oimi, None,
                                      A.mult)
                    dve.scalar_tensor_tensor(fg[:], oreb[:, w0:w0 + FW], orei,
                                             fg[:], A.mult, A.add)
                    dve.tensor_scalar(fm1[:], fsq[:], float(RR2), None, A.is_le)
                    dve.tensor_scalar(fm2[:], fg[:], 0.0, None, A.is_gt)
                    dve.tensor_tensor(fwrr[:], fm1[:], fm2[:], A.mult)
                    dve.tensor_scalar(fwro[:], fsq[:], float(RO2), None, A.is_le)
                    dve.tensor_tensor(fwro[:, BF:BF + 128],
                                      fwro[:, BF:BF + 128], eye[:], A.max)

                    dve.tensor_scalar(fwrr[:], fwrr[:], 1.0, None, A.mult,
                                      A.add, accum_out=nrt_[:, t:t + 1])
                    dve.scalar_tensor_tensor(ft1[:], fwrr[:], 1.0,
                                             pxb[:, w0:w0 + FW],
                                             A.mult, A.mult,
                                             accum_out=sxt[:, t:t + 1])
                    dve.scalar_tensor_tensor(ft2[:], fwrr[:], 1.0,
                                             pyb[:, w0:w0 + FW],
                                             A.mult, A.mult,
                                             accum_out=syt[:, t:t + 1])
                    dve.scalar_tensor_tensor(fdx[:], fwro[:], 1.0,
                                             oreb[:, w0:w0 + FW],
                                             A.mult, A.mult,
                                             accum_out=oxt[:, t:t + 1])
                    dve.scalar_tensor_tensor(fdy[:], fwro[:], 1.0,
                                             oimb[:, w0:w0 + FW],
                                             A.mult, A.mult,
                                             accum_out=oyt[:, t:t + 1])

            # ---------------- per-particle forward math [128, NT] --------
            def tt(name):
                return T([128, NT], name)

            c1x = T([1, 16], "c1x")
            c1y = T([1, 16], "c1y")
            cmsxb16 = T([128, 16], "cmsxb16")
            cmsyb16 = T([128, 16], "cmsyb16")
            dve.memset(c1x[:], 0.0)
            dve.memset(c1y[:], 0.0)
            dve.tensor_reduce(c1x[:, 0:1], fpx[:], mybir.AxisListType.X, A.add)
            dve.tensor_reduce(c1y[:, 0:1], fpy[:], mybir.AxisListType.X, A.add)
            dve.tensor_scalar(c1x[:], c1x[:], float(F32(1.0 / N)), None, A.mult)
            dve.tensor_scalar(c1y[:], c1y[:], float(F32(1.0 / N)), None, A.mult)
            gps.partition_broadcast(cmsxb16[:], c1x[:])
            gps.partition_broadcast(cmsyb16[:], c1y[:])
            cmsxb = cmsxb16[:, 0:1]
            cmsyb = cmsyb16[:, 0:1]

            nrx = tt("nrx")
            inr = tt("inr")
            snr = tt("snr")
            dve.tensor_scalar(nrx[:], nrt_[:], 1.0, None, A.max)
            dve.reciprocal(inr[:], nrx[:])
            dve.tensor_scalar(snr[:], nrt_[:], 0.0, None, A.is_gt)
            dxv = tt("dxv")
            dyv = tt("dyv")
            w1 = tt("w1")
            w2 = tt("w2")
            dve.tensor_tensor(w1[:], sxt[:], inr[:], A.mult)
            dve.tensor_tensor(w2[:], ow(0), snr[:], A.mult)
            dve.tensor_tensor(dxv[:], w2[:], w1[:], A.subtract)
            dve.tensor_tensor(w1[:], syt[:], inr[:], A.mult)
            dve.tensor_tensor(w2[:], ow(1), snr[:], A.mult)
            dve.tensor_tensor(dyv[:], w2[:], w1[:], A.subtract)

            psx = tt("psx")
            psy = tt("psy")
            dve.tensor_scalar(psx[:], ow(0), cmsxb, -1.0,
                              A.subtract, A.mult)
            dve.tensor_scalar(psy[:], ow(1), cmsyb, -1.0,
                              A.subtract, A.mult)

            cd = tt("cd")
            sd = tt("sd")
            act.activation(cd[:], ow(4), AF.Sin, bias=cbias[:])
            act.activation(sd[:], ow(4), AF.Sin)
            ta = tt("ta")
            tb = tt("tb")
            tc_ = tt("tc_")
            td = tt("td")
            dve.tensor_tensor(ta[:], psx[:], cd[:], A.mult)
            dve.tensor_tensor(tb[:], psy[:], sd[:], A.mult)
            dve.tensor_tensor(tc_[:], psy[:], cd[:], A.mult)
            dve.tensor_tensor(td[:], psx[:], sd[:], A.mult)
            lx = tt("lx")
            ly = tt("ly")
            rx = tt("rx")
            ry = tt("ry")
            dve.tensor_tensor(lx[:], ta[:], tb[:], A.subtract)
            dve.tensor_tensor(ly[:], tc_[:], td[:], A.add)
            dve.tensor_tensor(rx[:], ta[:], tb[:], A.add)
            dve.tensor_tensor(ry[:], tc_[:], td[:], A.subtract)

            dl = tt("dl")
            dr_ = tt("dr_")
            dve.tensor_tensor(w1[:], lx[:], oxt[:], A.mult)
            dve.tensor_tensor(w2[:], ly[:], oyt[:], A.mult)
            dve.tensor_tensor(dl[:], w1[:], w2[:], A.add)
            dve.tensor_tensor(w1[:], rx[:], oxt[:], A.mult)
            dve.tensor_tensor(w2[:], ry[:], oyt[:], A.mult)
            dve.tensor_tensor(dr_[:], w1[:], w2[:], A.add)
            mlr = tt("mlr")
            mlrc = tt("mlrc")
            dve.tensor_tensor(w1[:], dl[:], dr_[:], A.subtract)
            dve.tensor_scalar(mlr[:], w1[:], 0.0, None, A.is_ge)
            dve.tensor_scalar(mlrc[:], mlr[:], -1.0, 1.0, A.mult, A.add)
            bx = tt("bx")
            by = tt("by")
            dve.tensor_tensor(w1[:], mlr[:], lx[:], A.mult)
            dve.tensor_tensor(w2[:], mlrc[:], rx[:], A.mult)
            dve.tensor_tensor(bx[:], w1[:], w2[:], A.add)
            dve.tensor_tensor(w1[:], mlr[:], ly[:], A.mult)
            dve.tensor_tensor(w2[:], mlrc[:], ry[:], A.mult)
            dve.tensor_tensor(by[:], w1[:], w2[:], A.add)

            hr = tt("hr")
            dve.tensor_tensor(w1[:], dxv[:], dxv[:], A.mult)
            dve.tensor_tensor(w2[:], dyv[:], dyv[:], A.mult)
            dve.tensor_tensor(w1[:], w1[:], w2[:], A.add)
            dve.tensor_scalar(hr[:], w1[:], 0.0, None, A.is_gt)
            hrc = tt("hrc")
            dve.tensor_scalar(hrc[:], hr[:], -1.0, 1.0, A.mult, A.add)
            zx = tt("zx")
            zy = tt("zy")
            dve.tensor_tensor(w1[:], hr[:], dxv[:], A.mult)
            dve.tensor_tensor(w2[:], hrc[:], bx[:], A.mult)
            dve.tensor_tensor(zx[:], w1[:], w2[:], A.add)
            dve.tensor_tensor(w1[:], hr[:], dyv[:], A.mult)
            dve.tensor_tensor(w2[:], hrc[:], by[:], A.mult)
            dve.tensor_tensor(zy[:], w1[:], w2[:], A.add)
            wx = tt("wx")
            wy = tt("wy")
            dve.tensor_tensor(w1[:], zx[:], ow(2), A.mult)
            dve.tensor_tensor(w2[:], zy[:], ow(3), A.mult)
            dve.tensor_tensor(wx[:], w1[:], w2[:], A.add)
            dve.tensor_tensor(w1[:], zy[:], ow(2), A.mult)
            dve.tensor_tensor(w2[:], zx[:], ow(3), A.mult)
            dve.tensor_tensor(wy[:], w1[:], w2[:], A.subtract)
            # sin(att) = sin(angle(z1) - angle(ori)) = wy / |w|
            satt = tt("satt")
            act.activation(w1[:], wx[:], AF.Square)
            act.activation(w2[:], wy[:], AF.Square)
            dve.tensor_tensor(w1[:], w1[:], w2[:], A.add)
            dve.tensor_scalar(w1[:], w1[:], 1.0e-37, None, A.max)
            dve.reciprocal(w1[:], w1[:])
            act.activation(w1[:], w1[:], AF.Sqrt)
            dve.tensor_tensor(satt[:], wy[:], w1[:], A.mult)
            th = tt("th")
            dve.tensor_scalar(w1[:], satt[:], float(C_GGD), None, A.mult)
            dve.scalar_tensor_tensor(th[:], ow(5), float(C_RN), w1[:], A.mult, A.add)
            cth = tt("cth")
            sth = tt("sth")
            act.activation(cth[:], th[:], AF.Sin, bias=cbias[:])
            act.activation(sth[:], th[:], AF.Sin)
            nox = tt("nox")
            noy = tt("noy")
            dve.tensor_tensor(w1[:], ow(2), cth[:], A.mult)
            dve.tensor_tensor(w2[:], ow(3), sth[:], A.mult)
            dve.tensor_tensor(nox[:], w1[:], w2[:], A.subtract)
            dve.tensor_tensor(w1[:], ow(3), cth[:], A.mult)
            dve.tensor_tensor(w2[:], ow(2), sth[:], A.mult)
            dve.tensor_tensor(noy[:], w1[:], w2[:], A.add)

            dma.dma_start(o_ori[:, :, 0], nox[:])
            dma.dma_start(o_ori[:, :, 1], noy[:])
            dma.dma_start(o_osum[:, :, 0], oxt[:])
            dma.dma_start(o_osum[:, :, 1], oyt[:])
            dma.dma_start(o_left[:, :, 0], lx[:])
            dma.dma_start(o_left[:, :, 1], ly[:])
            dma.dma_start(o_right[:, :, 0], rx[:])
            dma.dma_start(o_right[:, :, 1], ry[:])

            # =====================================================
            # COLLISION LOOP (30 iterations, band BC)
            # =====================================================
            cpxb = T([128, CWIN], "cpxb")
            cpyb = T([128, CWIN], "cpyb")
            npx = T([128, NT], "npx")
            npy = T([128, NT], "npy")
            tmpd = T([128, NT], "tmpd")

            with tc.tile_pool(name="csb", bufs=3) as cp:
                fx = cp.tile([1, NPAD], f32, name="fx", tag="fx", bufs=1)
                fy = cp.tile([1, NPAD], f32, name="fy", tag="fy", bufs=1)
                dve.memset(fx[:], float(SENT))
                dve.memset(fy[:], float(SENT))
                prev_agout = None
                woff_h = []
                for it in range(COLL_ITERS):
                    if it > 0:
                        if not woff_h:
                            woff_h.append(dve.snap(
                                dve.partition_id() * ROWS, donate=False))
                        woff = woff_h[0]
                        dma.dma_start(fx[0:1, PAD:PAD + N], prev_agout[:, 0:1])
                        dma.dma_start(fy[0:1, PAD:PAD + N], prev_agout[:, 1:2])
                        dve.tensor_copy(cwx[0:1, 0:CWIN],
                                        fx[0:1, bass.ds(woff, CWIN)])
                        dve.tensor_copy(cwy[0:1, 0:CWIN],
                                        fy[0:1, bass.ds(woff, CWIN)])
                    gps.partition_broadcast(cpxb[:], cwx[:])
                    gps.partition_broadcast(cpyb[:], cwy[:])

                    for t in range(NT):
                        w0 = 128 * t
                        cdx = cp.tile([128, CW], f32, tag="cdx")
                        cdy = cp.tile([128, CW], f32, tag="cdy")
                        ct1 = cp.tile([128, CW], f32, tag="ct1")
                        ct2 = cp.tile([128, CW], f32, tag="ct2")
                        csq = cp.tile([128, CW], f32, tag="csq")
                        cr2 = cp.tile([128, CW], f32, tag="cr2")
                        crv = cp.tile([128, CW], f32, tag="crv")
                        cm = cp.tile([128, CW], f32, tag="cm")
                        ch = cp.tile([128, CW], f32, tag="ch")
                        cfm = cp.tile([128, CW], f32, tag="cfm")
                        cmx = cp.tile([128, CW], f32, tag="cmx")
                        cmy = cp.tile([128, CW], f32, tag="cmy")

                        dve.tensor_scalar(cdx[:], cpxb[:, w0:w0 + CW],
                                          px[:, t:t + 1], None, A.subtract)
                        dve.tensor_scalar(cdy[:], cpyb[:, w0:w0 + CW],
                                          py[:, t:t + 1], None, A.subtract)
                        dve.tensor_tensor(cdx[:, BC:BC + 128],
                                          cdx[:, BC:BC + 128], eye[:], A.add)
                        act.activation(ct1[:], cdx[:], AF.Square)
                        act.activation(ct2[:], cdy[:], AF.Square)
                        dve.tensor_tensor(csq[:], ct1[:], ct2[:], A.add)
                        dve.tensor_scalar(cm[:], csq[:], float(C2), None, A.is_le)
                        dve.reciprocal(cr2[:], csq[:])
                        act.activation(crv[:], cr2[:], AF.Sqrt)
                        dve.tensor_scalar(ch[:], crv[:], float(FAC), -0.5,
                                          A.mult, A.add)
                        dve.tensor_tensor(cfm[:], cm[:], ch[:], A.mult)
                        dve.scalar_tensor_tensor(
                            cmx[:], cdx[:], 1.0, cfm[:], A.mult, A.mult,
                            accum_out=sumx[:, t:t + 1])
                        dve.scalar_tensor_tensor(
                            cmy[:], cdy[:], 1.0, cfm[:], A.mult, A.mult,
                            accum_out=sumy[:, t:t + 1])

                    dve.tensor_tensor(npx[:], px[:], sumx[:], A.subtract)
                    dve.tensor_tensor(npy[:], py[:], sumy[:], A.subtract)
                    dve.tensor_copy(px[:], npx[:])
                    dve.tensor_copy(py[:], npy[:])

                    dve.tensor_tensor(tmpd[:], px[:], px0[:], A.subtract)
                    dve.tensor_tensor(tmpd[:], tmpd[:], tmpd[:], A.mult)
                    dve.tensor_tensor(gx[:], gx[:], tmpd[:], A.max)
                    dve.tensor_tensor(tmpd[:], py[:], py0[:], A.subtract)
                    dve.tensor_tensor(tmpd[:], tmpd[:], tmpd[:], A.mult)
                    dve.tensor_tensor(gy[:], gy[:], tmpd[:], A.max)

                    if it < COLL_ITERS - 1:
                        agin = dp.tile([ROWS, 2], f32, name="agin",
                                       tag="agin", bufs=2)
                        agout = dp.tile([N, 2], f32, name="agout",
                                        tag="agout", bufs=2,
                                        addr_space="Shared")
                        agr = agin[:].rearrange("(t p) c -> p t c", p=128)
                        dma.dma_start(agr[:, :, 0], px[:])
                        dma.dma_start(agr[:, :, 1], py[:])
                        gps.collective_compute(
                            "AllGather", A.bypass, replica_groups=rg,
                            ins=[agin[:].opt()],
                            outs=[agout[:].opt()])
                        prev_agout = agout

            dma.dma_start(o_pos[:, :, 0], px[:])
            dma.dma_start(o_pos[:, :, 1], py[:])
            dma.dma_start(o_guard[:, :, 0], gx[:])
            dma.dma_start(o_guard[:, :, 1], gy[:])

    nc.compile()
    return nc


# =====================================================================
# Host-side helpers
# =====================================================================
def _win(arr_s, c, band, fill):
    """arr_s padded window [c*ROWS - band, c*ROWS + ROWS + band)."""
    lo = c * ROWS - band
    hi = c * ROWS + ROWS + band
    out = np.full(hi - lo, fill, dtype=np.float32)
    a, b = max(lo, 0), min(hi, N)
    out[a - lo:b - lo] = arr_s[a:b]
    return out


def _prepare_inputs(ins_s, p0x_s, p0y_s):
    """Per-core input dicts from sorted arrays."""
    names = ["pos_re", "pos_im", "ori_re", "ori_im", "deltas", "rot_noise",
             "tn_re", "tn_im"]
    eye = np.eye(128, dtype=np.float32)
    fpx = ins_s["pos_re"][None, :].copy()
    fpy = ins_s["pos_im"][None, :].copy()
    in_maps = []
    for c in range(NCORES):
        sl = slice(c * ROWS, (c + 1) * ROWS)
        own = np.stack([ins_s[k][sl] for k in names], axis=-1)  # [512, 8]
        own = own.reshape(NT, 128, 8).transpose(1, 0, 2).copy()  # [128,4,8]

        cwin = np.stack([
            _win(ins_s["pos_re"], c, BC, SENT),
            _win(ins_s["pos_im"], c, BC, SENT),
            _win(ins_s["ori_re"], c, BC, 0.0),
            _win(ins_s["ori_im"], c, BC, 0.0),
            _win(ins_s["tn_re"], c, BC, 0.0),
            _win(ins_s["tn_im"], c, BC, 0.0),
        ])
        fwin = np.stack([
            _win(ins_s["pos_re"], c, BF, SENT),
            _win(ins_s["pos_im"], c, BF, SENT),
            _win(ins_s["ori_re"], c, BF, 0.0),
            _win(ins_s["ori_im"], c, BF, 0.0),
        ])
        in_maps.append({
            "own8": own, "cwin": cwin, "fwin": fwin, "fpx": fpx, "fpy": fpy,
            "eye128": eye,
        })
    return in_maps


def _reference_numpy(pos_re, pos_im, ori_re, ori_im, deltas, rot_noise,
                     tn_re, tn_im):
    """Dense fp32 fallback with exact reference semantics."""
    pos = (pos_re + 1j * pos_im).astype(np.complex64)
    ori = (ori_re + 1j * ori_im).astype(np.complex64)
    n = pos.shape[0]
    eye_b = np.eye(n, dtype=bool)
    eye_f = np.eye(n, dtype=np.float32)
    eye_c = eye_f.astype(np.complex64)

    dists = np.abs(pos[:, None] - pos[None, :] + eye_c).astype(np.float32)
    mask_rr = (dists <= RR) & ~eye_b
    mask_ro = (dists <= RO + RC) | eye_b

    def wrap(diff):
        diff = np.where(diff <= -PI, np.mod(diff, np.float32(PI)), diff)
        return diff - (diff >= PI).astype(np.float32) * np.float32(2.0 * PI)

    ang = np.angle(ori).astype(np.float32)
    abs_ad = np.abs(wrap(ang[:, None] - ang[None, :]))
    in_front = abs_ad < PI / 2

    Wrr = (mask_rr & in_front).astype(np.float32)
    Wro = mask_ro.astype(np.float32)

    def mvc(W, z):
        return (W @ z.real.astype(np.float32)
                + 1j * (W @ z.imag.astype(np.float32))).astype(np.complex64)

    n_r = Wrr.sum(axis=1)
    S = mvc(Wrr, pos) / np.maximum(n_r, 1.0) - pos * np.sign(n_r)
    d = -S
    cms = pos.sum() / np.float32(n)
    Ps = (cms - pos).astype(np.complex64)
    osum = mvc(Wro, ori)
    left = Ps * np.exp(1j * deltas).astype(np.complex64)
    right = Ps * np.exp(-1j * deltas).astype(np.complex64)

    def cossim(a, b):
        dot = a.real * b.real + a.imag * b.imag
        na = np.maximum(np.abs(a), 1e-14)
        nb = np.maximum(np.abs(b), 1e-14)
        return dot / (na * nb)

    best = np.where(cossim(left, osum) >= cossim(right, osum), left, right)
    has_rep = np.abs(d) > 0.0
    z1 = np.where(has_rep, d, best)
    att = wrap(np.angle(z1).astype(np.float32) - np.angle(ori).astype(np.float32))
    s2dr = np.float32(np.sqrt(2.0 * DR))
    sdt = np.float32(np.sqrt(DT))
    rot = np.exp(1j * (DT * GAMMA * DR * np.sin(att)
                       + rot_noise * s2dr * sdt)).astype(np.complex64)
    tnoise = ((tn_re + 1j * tn_im) * np.float32(np.sqrt(0.5))
              * np.float32(np.sqrt(2.0 * DT_TRANS))).astype(np.complex64)
    trans = (np.float32(DT * VEL) * ori + tnoise * sdt).astype(np.complex64)

    p = (pos + trans).astype(np.complex64)
    cont = True
    for _ in range(COLL_ITERS):
        diff = p[None, :] - p[:, None] + eye_c
        absd = np.abs(diff).astype(np.float32)
        coll = absd <= (2.0 * RC - eye_f)
        move = np.where(coll,
                        diff / np.where(coll, absd, 1.0)
                        * ((2.1 * RC - absd) * 0.5), 0.0)
        if cont:
            p = (p - move.sum(axis=1)).astype(np.complex64)
            cont = bool(coll.sum() > 0)
    new_pos = p
    new_ori = (ori * rot).astype(np.complex64)

    def c2r(z):
        return np.stack([z.real, z.imag], axis=-1).astype(np.float32)

    return np.stack([c2r(new_pos), c2r(new_ori), c2r(osum), c2r(left),
                     c2r(right)], axis=0)


def _asm(res, key):
    """[128, NT, 2] per-core outputs -> [N, 2] sorted order."""
    out = np.empty((N, 2), np.float32)
    for c in range(NCORES):
        a = res[c][key]  # [128, NT, 2]
        out[c * ROWS:(c + 1) * ROWS] = a.transpose(1, 0, 2).reshape(ROWS, 2)
    return out


def kernel(pos_re, pos_im, ori_re, ori_im, deltas, rot_noise, tn_re, tn_im):
    global _BUILT, LAST_EXEC_NS
    import sys
    sys.path.insert(0, "/opt/trn_rl_repo")
    from concourse.bass_utils import run_bass_kernel_spmd

    ins = dict(pos_re=pos_re, pos_im=pos_im, ori_re=ori_re, ori_im=ori_im,
               deltas=deltas, rot_noise=rot_noise, tn_re=tn_re, tn_im=tn_im)
    ins = {k: np.asarray(v, np.float32) for k, v in ins.items()}

    # host: trans + sort permutation (schedule only; guarded below)
    trans_x = C_DV * ins["ori_re"] + C_TN * ins["tn_re"]
    trans_y = C_DV * ins["ori_im"] + C_TN * ins["tn_im"]
    p0x = ins["pos_re"] + trans_x
    p0y = ins["pos_im"] + trans_y
    perm = np.argsort(p0x, kind="stable")
    ins_s = {k: v[perm] for k, v in ins.items()}
    p0x_s, p0y_s = p0x[perm], p0y[perm]

    # ---- host-side band guards (exact) ----
    xs = ins_s["pos_re"].astype(np.float64)
    premax = np.maximum.accumulate(xs)
    sufmin = np.minimum.accumulate(xs[::-1])[::-1]
    ok_f = bool(np.all(sufmin[BF:] - premax[:-BF] > (RO + RC) * 1.001))
    gap128 = float(np.min(p0x_s[BC:].astype(np.float64)
                          - p0x_s[:-BC].astype(np.float64)))

    if not ok_f:
        out_full = _reference_numpy(**ins)
        return out_full

    if _BUILT is None:
        _BUILT = _build_nc()
    nc = _BUILT

    in_maps = _prepare_inputs(ins_s, p0x_s, p0y_s)
    r = run_bass_kernel_spmd(nc, in_maps, core_ids=list(range(NCORES)),
                             trace=TRACE)
    LAST_EXEC_NS = r.exec_time_ns
    res = r.results

    # collision guard: max displacement vs sorted-gap bound
    dmax = 0.0
    for c in range(NCORES):
        g = res[c]["o_guard"].astype(np.float64)
        dmax = max(dmax, float(np.max(np.sqrt(g[:, :, 0] + g[:, :, 1]))))
    gap_wrap = float(p0x_s[3584].astype(np.float64) - p0x_s[511].astype(np.float64))
    if not (gap128 > 2.0 * RC * 1.001 + 2.0 * dmax + 1e-8
            and gap_wrap > 2.0 * RC * 1.001 + 2.0 * dmax + 1e-8):
        return _reference_numpy(**ins)

    out_s = np.stack([_asm(res, "o_pos"), _asm(res, "o_ori"),
                      _asm(res, "o_osum"), _asm(res, "o_left"),
                      _asm(res, "o_right")], axis=0)
    out = np.empty_like(out_s)
    out[:, perm, :] = out_s
    return out


# revision 28
# speedup vs baseline: 1.5863x; 1.5863x over previous
"""Trainium2 Bass kernel for the ActiveParticles nn.Module (N=4096, 8 cores).

Strategy
--------
Host sorts particles by post-translation x coordinate (a pure permutation —
the reference computation is permutation-equivariant). In sorted space every
interaction radius becomes a *rank band*:
  - collision radius 2*RC  -> band +-128 ranks   (30-iteration loop)
  - orientation radius RO+RC -> band +-384 ranks (one-time masks)
Each of the 8 cores owns 512 consecutive sorted rows and computes only a
[128 x band] tile strip per row-tile. The collision loop allgathers the
updated 512x2 positions every iteration. Band validity is *proved* per input:
host-side exact gap checks + a device-computed max-displacement guard; if any
check fails, a dense numpy fallback (exact reference semantics) is used.

Outputs are produced in sorted space and unpermuted on host.
"""

import os
import numpy as np

# ---------------------------------------------------------------- constants
PI = 3.1415927410125732
N = 4096
L = 1.28e-3
VEL = 5e-7
RC = 3.15e-6
RR = 8e-6
RO = 2.5e-5
DT_TRANS = 1.4e-14
DR = 0.0028
GAMMA = 25.0
DT = 0.2
COLL_ITERS = 30

NCORES = 8
ROWS = N // NCORES          # 512 rows per core
NT = ROWS // 128            # 4 row-tiles per core

BC = 128                    # collision band (ranks each side)
CW = 2 * BC + 128           # collision tile window = 384
CWIN = ROWS + 2 * BC        # per-core collision window = 768

BF = 384                    # forward (mask) band
FW = 2 * BF + 128           # forward tile window = 896
FWIN = ROWS + 2 * BF        # per-core forward window = 1280

PAD = 128                   # sentinel pad rows each side of the shared array
NPAD = N + 2 * PAD          # 4352
SENT = np.float32(1.0e9)

F32 = np.float32
C2 = F32((2.0 * RC) ** 2)          # collision threshold on squared dist
FAC = F32(1.05 * RC)               # (2.1*RC)*0.5
RR2 = F32(RR ** 2)
RO2 = F32((RO + RC) ** 2)
C_DV = F32(DT * VEL)
C_TN = F32(np.sqrt(0.5) * np.sqrt(2.0 * DT_TRANS) * np.sqrt(DT))
C_GGD = F32(DT * GAMMA * DR)
C_RN = F32(np.sqrt(2.0 * DR) * np.sqrt(DT))
HALF_PI = F32(PI / 2)

_BUILT = None
TRACE = False               # test.py sets True to collect exec_time_ns
LAST_EXEC_NS = None


# =====================================================================
# Device program
# =====================================================================
def _build_nc():
    import concourse.bass as bass
    import concourse.mybir as mybir
    from concourse import tile, bass_isa, library_config
    from concourse import bacc

    f32 = mybir.dt.float32
    A = mybir.AluOpType
    AF = mybir.ActivationFunctionType

    nc = bacc.Bacc("TRN2", target_bir_lowering=False, debug=False,
                   num_devices=NCORES)

    # ---------------- I/O ----------------
    t_own = nc.dram_tensor("own8", [128, NT, 8], f32, kind="ExternalInput")
    t_cwin = nc.dram_tensor("cwin", [6, CWIN], f32, kind="ExternalInput")
    t_fwin = nc.dram_tensor("fwin", [4, FWIN], f32, kind="ExternalInput")
    t_fpx = nc.dram_tensor("fpx", [1, N], f32, kind="ExternalInput")
    t_fpy = nc.dram_tensor("fpy", [1, N], f32, kind="ExternalInput")
    t_eye = nc.dram_tensor("eye128", [128, 128], f32, kind="ExternalInput")

    o_pos = nc.dram_tensor("o_pos", [128, NT, 2], f32, kind="ExternalOutput")
    o_ori = nc.dram_tensor("o_ori", [128, NT, 2], f32, kind="ExternalOutput")
    o_osum = nc.dram_tensor("o_osum", [128, NT, 2], f32, kind="ExternalOutput")
    o_left = nc.dram_tensor("o_left", [128, NT, 2], f32, kind="ExternalOutput")
    o_right = nc.dram_tensor("o_right", [128, NT, 2], f32, kind="ExternalOutput")
    o_guard = nc.dram_tensor("o_guard", [128, NT, 2], f32, kind="ExternalOutput")

    rg = [list(range(NCORES))]

    with tile.TileContext(nc) as tc:
        dve, act, gps, dma = nc.vector, nc.scalar, nc.gpsimd, nc.sync
        with tc.tile_pool(name="main", bufs=1) as mp, \
             tc.tile_pool(name="dram", bufs=2, space="DRAM") as dp:

            def T(shape, name, dt=f32):
                return mp.tile(shape, dt, name=name, tag=name, bufs=1)

            # persistent SBUF state
            own = T([128, NT, 8], "own")
            cwx = T([1, CWIN], "cwx")
            cwy = T([1, CWIN], "cwy")
            cwo1 = T([1, CWIN], "cwo1")
            cwo2 = T([1, CWIN], "cwo2")
            cwt1 = T([1, CWIN], "cwt1")
            cwt2 = T([1, CWIN], "cwt2")
            fwx = T([1, FWIN], "fwx")
            fwy = T([1, FWIN], "fwy")
            fwo1 = T([1, FWIN], "fwo1")
            fwo2 = T([1, FWIN], "fwo2")
            eye = T([128, 128], "eye")
            fpx = T([1, N], "fpx")
            fpy = T([1, N], "fpy")
            px = T([128, NT], "px")
            py = T([128, NT], "py")
            px0 = T([128, NT], "px0")
            py0 = T([128, NT], "py0")
            gx = T([128, NT], "gx")
            gy = T([128, NT], "gy")
            sumx = T([128, NT], "sumx")
            sumy = T([128, NT], "sumy")
            nrt_ = T([128, NT], "nrt")
            sxt = T([128, NT], "sxt")
            syt = T([128, NT], "syt")
            oxt = T([128, NT], "oxt")
            oyt = T([128, NT], "oyt")
            cbias = T([128, 1], "cbias")

            dma.dma_start(own[:], t_own[:])
            dma.dma_start(cwx[:], t_cwin[0:1, :])
            dma.dma_start(cwy[:], t_cwin[1:2, :])
            dma.dma_start(cwo1[:], t_cwin[2:3, :])
            dma.dma_start(cwo2[:], t_cwin[3:4, :])
            dma.dma_start(cwt1[:], t_cwin[4:5, :])
            dma.dma_start(cwt2[:], t_cwin[5:6, :])
            dma.dma_start(fwx[:], t_fwin[0:1, :])
            dma.dma_start(fwy[:], t_fwin[1:2, :])
            dma.dma_start(fwo1[:], t_fwin[2:3, :])
            dma.dma_start(fwo2[:], t_fwin[3:4, :])
            dma.dma_start(eye[:], t_eye[:])
            dma.dma_start(fpx[:], t_fpx[:])
            dma.dma_start(fpy[:], t_fpy[:])
            dve.memset(cbias[:], float(HALF_PI))


            def ow(k):
                return own[:, :, k]

            # seed p0 = pos + trans (own rows)
            dve.scalar_tensor_tensor(px[:], ow(2), float(C_DV), ow(0), A.mult, A.add)
            dve.scalar_tensor_tensor(px[:], ow(6), float(C_TN), px[:], A.mult, A.add)
            dve.scalar_tensor_tensor(py[:], ow(3), float(C_DV), ow(1), A.mult, A.add)
            dve.scalar_tensor_tensor(py[:], ow(7), float(C_TN), py[:], A.mult, A.add)
            dve.tensor_copy(px0[:], px[:])
            dve.tensor_copy(py0[:], py[:])
            dve.memset(gx[:], 0.0)
            dve.memset(gy[:], 0.0)

            # seed window p0 = pos + trans
            dve.scalar_tensor_tensor(cwx[:], cwo1[:], float(C_DV), cwx[:],
                                     A.mult, A.add)
            dve.scalar_tensor_tensor(cwx[:], cwt1[:], float(C_TN), cwx[:],
                                     A.mult, A.add)
            dve.scalar_tensor_tensor(cwy[:], cwo2[:], float(C_DV), cwy[:],
                                     A.mult, A.add)
            dve.scalar_tensor_tensor(cwy[:], cwt2[:], float(C_TN), cwy[:],
                                     A.mult, A.add)

            # =====================================================
            # FORWARD one-time: masks + reductions (band BF)
            # =====================================================
            with tc.tile_pool(name="fsb", bufs=2) as fp:
                pxb = fp.tile([128, FWIN], f32, name="pxb", tag="pxb", bufs=1)
                pyb = fp.tile([128, FWIN], f32, name="pyb", tag="pyb", bufs=1)
                oreb = fp.tile([128, FWIN], f32, name="oreb", tag="oreb", bufs=1)
                oimb = fp.tile([128, FWIN], f32, name="oimb", tag="oimb", bufs=1)
                gps.partition_broadcast(pxb[:], fwx[:])
                gps.partition_broadcast(pyb[:], fwy[:])
                gps.partition_broadcast(oreb[:], fwo1[:])
                gps.partition_broadcast(oimb[:], fwo2[:])

                for t in range(NT):
                    w0 = 128 * t
                    pxi = own[:, t, 0:1]
                    pyi = own[:, t, 1:2]
                    orei = own[:, t, 2:3]
                    oimi = own[:, t, 3:4]
                    fdx = fp.tile([128, FW], f32, tag="fdx")
                    fdy = fp.tile([128, FW], f32, tag="fdy")
                    ft1 = fp.tile([128, FW], f32, tag="ft1")
                    ft2 = fp.tile([128, FW], f32, tag="ft2")
                    fsq = fp.tile([128, FW], f32, tag="fsq")
                    fg = fp.tile([128, FW], f32, tag="fg")
                    fm1 = fp.tile([128, FW], f32, tag="fm1")
                    fm2 = fp.tile([128, FW], f32, tag="fm2")
                    fwrr = fp.tile([128, FW], f32, tag="fwrr")
                    fwro = fp.tile([128, FW], f32, tag="fwro")

                    dve.tensor_scalar(fdx[:], pxb[:, w0:w0 + FW], pxi, None,
                                      A.subtract)
                    dve.tensor_scalar(fdy[:], pyb[:, w0:w0 + FW], pyi, None,
                                      A.subtract)
                    dve.tensor_tensor(fdx[:, BF:BF + 128], fdx[:, BF:BF + 128],
                                      eye[:], A.add)
                    act.activation(ft1[:], fdx[:], AF.Square)
                    act.activation(ft2[:], fdy[:], AF.Square)
                    dve.tensor_tensor(fsq[:], ft1[:], ft2[:], A.add)
                    dve.tensor_scalar(fg[:], oimb[:, w0:w0 + FW], oimi, None,
                                      A.mult)
                    dve.scalar_tensor_tensor(fg[:], oreb[:, w0:w0 + FW], orei,
                                             fg[:], A.mult, A.add)
                    dve.tensor_scalar(fm1[:], fsq[:], float(RR2), None, A.is_le)
                    dve.tensor_scalar(fm2[:], fg[:], 0.0, None, A.is_gt)
                    dve.tensor_tensor(fwrr[:], fm1[:], fm2[:], A.mult)
                    dve.tensor_scalar(fwro[:], fsq[:], float(RO2), None, A.is_le)
                    dve.tensor_tensor(fwro[:, BF:BF + 128],
                                      fwro[:, BF:BF + 128], eye[:], A.max)

                    dve.tensor_scalar(fwrr[:], fwrr[:], 1.0, None, A.mult,
                                      A.add, accum_out=nrt_[:, t:t + 1])
                    dve.scalar_tensor_tensor(ft1[:], fwrr[:], 1.0,
                                             pxb[:, w0:w0 + FW],
                                             A.mult, A.mult,
                                             accum_out=sxt[:, t:t + 1])
                    dve.scalar_tensor_tensor(ft2[:], fwrr[:], 1.0,
                                             pyb[:, w0:w0 + FW],
                                             A.mult, A.mult,
                                             accum_out=syt[:, t:t + 1])
                    dve.scalar_tensor_tensor(fdx[:], fwro[:], 1.0,
                                             oreb[:, w0:w0 + FW],
                                             A.mult, A.mult,
                                             accum_out=oxt[:, t:t + 1])
                    dve.scalar_tensor_tensor(fdy[:], fwro[:], 1.0,
                                             oimb[:, w0:w0 + FW],
                                             A.mult, A.mult,
                                             accum_out=oyt[:, t:t + 1])

            # ---------------- per-particle forward math [128, NT] --------
            def tt(name):
                return T([128, NT], name)

            c1x = T([1, 16], "c1x")
            c1y = T([1, 16], "c1y")
            cmsxb16 = T([128, 16], "cmsxb16")
            cmsyb16 = T([128, 16], "cmsyb16")
            dve.memset(c1x[:], 0.0)
            dve.memset(c1y[:], 0.0)
            dve.tensor_reduce(c1x[:, 0:1], fpx[:], mybir.AxisListType.X, A.add)
            dve.tensor_reduce(c1y[:, 0:1], fpy[:], mybir.AxisListType.X, A.add)
            dve.tensor_scalar(c1x[:], c1x[:], float(F32(1.0 / N)), None, A.mult)
            dve.tensor_scalar(c1y[:], c1y[:], float(F32(1.0 / N)), None, A.mult)
            gps.partition_broadcast(cmsxb16[:], c1x[:])
            gps.partition_broadcast(cmsyb16[:], c1y[:])
            cmsxb = cmsxb16[:, 0:1]
            cmsyb = cmsyb16[:, 0:1]

            nrx = tt("nrx")
            inr = tt("inr")
            snr = tt("snr")
            dve.tensor_scalar(nrx[:], nrt_[:], 1.0, None, A.max)
            dve.reciprocal(inr[:], nrx[:])
            dve.tensor_scalar(snr[:], nrt_[:], 0.0, None, A.is_gt)
            dxv = tt("dxv")
            dyv = tt("dyv")
            w1 = tt("w1")
            w2 = tt("w2")
            dve.tensor_tensor(w1[:], sxt[:], inr[:], A.mult)
            dve.tensor_tensor(w2[:], ow(0), snr[:], A.mult)
            dve.tensor_tensor(dxv[:], w2[:], w1[:], A.subtract)
            dve.tensor_tensor(w1[:], syt[:], inr[:], A.mult)
            dve.tensor_tensor(w2[:], ow(1), snr[:], A.mult)
            dve.tensor_tensor(dyv[:], w2[:], w1[:], A.subtract)

            psx = tt("psx")
            psy = tt("psy")
            dve.tensor_scalar(psx[:], ow(0), cmsxb, -1.0,
                              A.subtract, A.mult)
            dve.tensor_scalar(psy[:], ow(1), cmsyb, -1.0,
                              A.subtract, A.mult)

            cd = tt("cd")
            sd = tt("sd")
            act.activation(cd[:], ow(4), AF.Sin, bias=cbias[:])
            act.activation(sd[:], ow(4), AF.Sin)
            ta = tt("ta")
            tb = tt("tb")
            tc_ = tt("tc_")
            td = tt("td")
            dve.tensor_tensor(ta[:], psx[:], cd[:], A.mult)
            dve.tensor_tensor(tb[:], psy[:], sd[:], A.mult)
            dve.tensor_tensor(tc_[:], psy[:], cd[:], A.mult)
            dve.tensor_tensor(td[:], psx[:], sd[:], A.mult)
            lx = tt("lx")
            ly = tt("ly")
            rx = tt("rx")
            ry = tt("ry")
            dve.tensor_tensor(lx[:], ta[:], tb[:], A.subtract)
            dve.tensor_tensor(ly[:], tc_[:], td[:], A.add)
            dve.tensor_tensor(rx[:], ta[:], tb[:], A.add)
            dve.tensor_tensor(ry[:], tc_[:], td[:], A.subtract)

            dl = tt("dl")
            dr_ = tt("dr_")
            dve.tensor_tensor(w1[:], lx[:], oxt[:], A.mult)
            dve.tensor_tensor(w2[:], ly[:], oyt[:], A.mult)
            dve.tensor_tensor(dl[:], w1[:], w2[:], A.add)
            dve.tensor_tensor(w1[:], rx[:], oxt[:], A.mult)
            dve.tensor_tensor(w2[:], ry[:], oyt[:], A.mult)
            dve.tensor_tensor(dr_[:], w1[:], w2[:], A.add)
            mlr = tt("mlr")
            mlrc = tt("mlrc")
            dve.tensor_tensor(w1[:], dl[:], dr_[:], A.subtract)
            dve.tensor_scalar(mlr[:], w1[:], 0.0, None, A.is_ge)
            dve.tensor_scalar(mlrc[:], mlr[:], -1.0, 1.0, A.mult, A.add)
            bx = tt("bx")
            by = tt("by")
            dve.tensor_tensor(w1[:], mlr[:], lx[:], A.mult)
            dve.tensor_tensor(w2[:], mlrc[:], rx[:], A.mult)
            dve.tensor_tensor(bx[:], w1[:], w2[:], A.add)
            dve.tensor_tensor(w1[:], mlr[:], ly[:], A.mult)
            dve.tensor_tensor(w2[:], mlrc[:], ry[:], A.mult)
            dve.tensor_tensor(by[:], w1[:], w2[:], A.add)

            hr = tt("hr")
            dve.tensor_tensor(w1[:], dxv[:], dxv[:], A.mult)
            dve.tensor_tensor(w2[:], dyv[:], dyv[:], A.mult)
            dve.tensor_tensor(w1[:], w1[:], w2[:], A.add)
            dve.tensor_scalar(hr[:], w1[:], 0.0, None, A.is_gt)
            hrc = tt("hrc")
            dve.tensor_scalar(hrc[:], hr[:], -1.0, 1.0, A.mult, A.add)
            zx = tt("zx")
            zy = tt("zy")
            dve.tensor_tensor(w1[:], hr[:], dxv[:], A.mult)
            dve.tensor_tensor(w2[:], hrc[:], bx[:], A.mult)
            dve.tensor_tensor(zx[:], w1[:], w2[:], A.add)
            dve.tensor_tensor(w1[:], hr[:], dyv[:], A.mult)
            dve.tensor_tensor(w2[:], hrc[:], by[:], A.mult)
            dve.tensor_tensor(zy[:], w1[:], w2[:], A.add)
            wx = tt("wx")
            wy = tt("wy")
            dve.tensor_tensor(w1[:], zx[:], ow(2), A.mult)
            dve.tensor_tensor(w2[:], zy[:], ow(3), A.mult)
            dve.tensor_tensor(wx[:], w1[:], w2[:], A.add)
            dve.tensor_tensor(w1[:], zy[:], ow(2), A.mult)
            dve.tensor_tensor(w2[:], zx[:], ow(3), A.mult)
            dve.tensor_tensor(wy[:], w1[:], w2[:], A.subtract)
            # sin(att) = sin(angle(z1) - angle(ori)) = wy / |w|
            satt = tt("satt")
            act.activation(w1[:], wx[:], AF.Square)
            act.activation(w2[:], wy[:], AF.Square)
            dve.tensor_tensor(w1[:], w1[:], w2[:], A.add)
            dve.tensor_scalar(w1[:], w1[:], 1.0e-37, None, A.max)
            dve.reciprocal(w1[:], w1[:])
            act.activation(w1[:], w1[:], AF.Sqrt)
            dve.tensor_tensor(satt[:], wy[:], w1[:], A.mult)
            th = tt("th")
            dve.tensor_scalar(w1[:], satt[:], float(C_GGD), None, A.mult)
            dve.scalar_tensor_tensor(th[:], ow(5), float(C_RN), w1[:], A.mult, A.add)
            cth = tt("cth")
            sth = tt("sth")
            act.activation(cth[:], th[:], AF.Sin, bias=cbias[:])
            act.activation(sth[:], th[:], AF.Sin)
            nox = tt("nox")
            noy = tt("noy")
            dve.tensor_tensor(w1[:], ow(2), cth[:], A.mult)
            dve.tensor_tensor(w2[:], ow(3), sth[:], A.mult)
            dve.tensor_tensor(nox[:], w1[:], w2[:], A.subtract)
            dve.tensor_tensor(w1[:], ow(3), cth[:], A.mult)
            dve.tensor_tensor(w2[:], ow(2), sth[:], A.mult)
            dve.tensor_tensor(noy[:], w1[:], w2[:], A.add)

            dma.dma_start(o_ori[:, :, 0], nox[:])
            dma.dma_start(o_ori[:, :, 1], noy[:])
            dma.dma_start(o_osum[:, :, 0], oxt[:])
            dma.dma_start(o_osum[:, :, 1], oyt[:])
            dma.dma_start(o_left[:, :, 0], lx[:])
            dma.dma_start(o_left[:, :, 1], ly[:])
            dma.dma_start(o_right[:, :, 0], rx[:])
            dma.dma_start(o_right[:, :, 1], ry[:])

            # =====================================================
            # COLLISION LOOP (30 iterations, band BC)
            # =====================================================
            cpxb = T([128, CWIN], "cpxb")
            cpyb = T([128, CWIN], "cpyb")
            npx = T([128, NT], "npx")
            npy = T([128, NT], "npy")
            tmpd = T([128, NT], "tmpd")

            with tc.tile_pool(name="csb", bufs=3) as cp:
                fx = cp.tile([1, NPAD], f32, name="fx", tag="fx", bufs=1)
                fy = cp.tile([1, NPAD], f32, name="fy", tag="fy", bufs=1)
                dve.memset(fx[:], float(SENT))
                dve.memset(fy[:], float(SENT))
                prev_agout = None
                woff_h = []
                for it in range(COLL_ITERS):
                    if it > 0:
                        if not woff_h:
                            woff_h.append(dve.snap(
                                dve.partition_id() * ROWS, donate=False))
                        woff = woff_h[0]
                        agx = prev_agout[:].rearrange("(r two) w -> two r w",
                                                      two=2)
                        dma.dma_start(fx[0:1, PAD:PAD + N], agx[0])
                        dma.dma_start(fy[0:1, PAD:PAD + N], agx[1])
                        dve.tensor_copy(cwx[0:1, 0:CWIN],
                                        fx[0:1, bass.ds(woff, CWIN)])
                        dve.tensor_copy(cwy[0:1, 0:CWIN],
                                        fy[0:1, bass.ds(woff, CWIN)])
                    gps.partition_broadcast(cpxb[:], cwx[:])
                    gps.partition_broadcast(cpyb[:], cwy[:])

                    cdx = cp.tile([128, NT, CW], f32, tag="cdx", bufs=1)
                    cdy = cp.tile([128, NT, CW], f32, tag="cdy", bufs=1)
                    ct1 = cp.tile([128, NT, CW], f32, tag="ct1", bufs=1)
                    ct2 = cp.tile([128, NT, CW], f32, tag="ct2", bufs=1)
                    csq = cp.tile([128, NT, CW], f32, tag="csq", bufs=1)
                    crv = cp.tile([128, NT, CW], f32, tag="crv", bufs=1)
                    cfm = cp.tile([128, NT, CW], f32, tag="cfm", bufs=1)
                    cmx = cp.tile([128, NT, CW], f32, tag="cmx", bufs=1)
                    for t in range(NT):
                        w0 = 128 * t
                        dve.tensor_scalar(cdx[:, t, :], cpxb[:, w0:w0 + CW],
                                          px[:, t:t + 1], None, A.subtract)
                        dve.tensor_scalar(cdy[:, t, :], cpyb[:, w0:w0 + CW],
                                          py[:, t:t + 1], None, A.subtract)
                        dve.tensor_tensor(cdx[:, t, BC:BC + 128],
                                          cdx[:, t, BC:BC + 128], eye[:], A.add)
                    act.activation(ct1[:], cdx[:], AF.Square)
                    act.activation(ct2[:], cdy[:], AF.Square)
                    dve.tensor_tensor(csq[:], ct1[:], ct2[:], A.add)
                    dve.reciprocal(ct1[:], csq[:])
                    act.activation(crv[:], ct1[:], AF.Sqrt)
                    dve.tensor_scalar(ct2[:], crv[:], float(FAC), -0.5,
                                      A.mult, A.add)
                    dve.scalar_tensor_tensor(cfm[:], csq[:], float(C2), ct2[:],
                                             A.is_le, A.mult)
                    for t in range(NT):
                        dve.scalar_tensor_tensor(
                            cmx[:, t, :], cdx[:, t, :], 1.0, cfm[:, t, :],
                            A.mult, A.mult, accum_out=sumx[:, t:t + 1])
                        dve.scalar_tensor_tensor(
                            cmx[:, t, :], cdy[:, t, :], 1.0, cfm[:, t, :],
                            A.mult, A.mult, accum_out=sumy[:, t:t + 1])

                    dve.tensor_tensor(npx[:], px[:], sumx[:], A.subtract)
                    dve.tensor_tensor(npy[:], py[:], sumy[:], A.subtract)
                    dve.tensor_copy(px[:], npx[:])
                    dve.tensor_copy(py[:], npy[:])

                    dve.tensor_tensor(tmpd[:], px[:], px0[:], A.subtract)
                    dve.tensor_tensor(tmpd[:], tmpd[:], tmpd[:], A.mult)
                    dve.tensor_tensor(gx[:], gx[:], tmpd[:], A.max)
                    dve.tensor_tensor(tmpd[:], py[:], py0[:], A.subtract)
                    dve.tensor_tensor(tmpd[:], tmpd[:], tmpd[:], A.mult)
                    dve.tensor_tensor(gy[:], gy[:], tmpd[:], A.max)

                    if it < COLL_ITERS - 1:
                        agin = dp.tile([2, ROWS], f32, name="agin",
                                       tag="agin", bufs=2)
                        agout = dp.tile([2 * NCORES, ROWS], f32, name="agout",
                                        tag="agout", bufs=2,
                                        addr_space="Shared")
                        agr = agin[:].rearrange("c (t p) -> c p t", p=128)
                        dma.dma_start(agr[0], px[:])
                        dma.dma_start(agr[1], py[:])
                        gps.collective_compute(
                            "AllGather", A.bypass, replica_groups=rg,
                            ins=[agin[:].opt()],
                            outs=[agout[:].opt()])
                        prev_agout = agout

            dma.dma_start(o_pos[:, :, 0], px[:])
            dma.dma_start(o_pos[:, :, 1], py[:])
            dma.dma_start(o_guard[:, :, 0], gx[:])
            dma.dma_start(o_guard[:, :, 1], gy[:])

    nc.compile()
    return nc


# =====================================================================
# Host-side helpers
# =====================================================================
def _win(arr_s, c, band, fill):
    """arr_s padded window [c*ROWS - band, c*ROWS + ROWS + band)."""
    lo = c * ROWS - band
    hi = c * ROWS + ROWS + band
    out = np.full(hi - lo, fill, dtype=np.float32)
    a, b = max(lo, 0), min(hi, N)
    out[a - lo:b - lo] = arr_s[a:b]
    return out


def _prepare_inputs(ins_s, p0x_s, p0y_s):
    """Per-core input dicts from sorted arrays."""
    names = ["pos_re", "pos_im", "ori_re", "ori_im", "deltas", "rot_noise",
             "tn_re", "tn_im"]
    eye = np.eye(128, dtype=np.float32)
    fpx = ins_s["pos_re"][None, :].copy()
    fpy = ins_s["pos_im"][None, :].copy()
    in_maps = []
    for c in range(NCORES):
        sl = slice(c * ROWS, (c + 1) * ROWS)
        own = np.stack([ins_s[k][sl] for k in names], axis=-1)  # [512, 8]
        own = own.reshape(NT, 128, 8).transpose(1, 0, 2).copy()  # [128,4,8]

        cwin = np.stack([
            _win(ins_s["pos_re"], c, BC, SENT),
            _win(ins_s["pos_im"], c, BC, SENT),
            _win(ins_s["ori_re"], c, BC, 0.0),
            _win(ins_s["ori_im"], c, BC, 0.0),
            _win(ins_s["tn_re"], c, BC, 0.0),
            _win(ins_s["tn_im"], c, BC, 0.0),
        ])
        fwin = np.stack([
            _win(ins_s["pos_re"], c, BF, SENT),
            _win(ins_s["pos_im"], c, BF, SENT),
            _win(ins_s["ori_re"], c, BF, 0.0),
            _win(ins_s["ori_im"], c, BF, 0.0),
        ])
        in_maps.append({
            "own8": own, "cwin": cwin, "fwin": fwin, "fpx": fpx, "fpy": fpy,
            "eye128": eye,
        })
    return in_maps


def _reference_numpy(pos_re, pos_im, ori_re, ori_im, deltas, rot_noise,
                     tn_re, tn_im):
    """Dense fp32 fallback with exact reference semantics."""
    pos = (pos_re + 1j * pos_im).astype(np.complex64)
    ori = (ori_re + 1j * ori_im).astype(np.complex64)
    n = pos.shape[0]
    eye_b = np.eye(n, dtype=bool)
    eye_f = np.eye(n, dtype=np.float32)
    eye_c = eye_f.astype(np.complex64)

    dists = np.abs(pos[:, None] - pos[None, :] + eye_c).astype(np.float32)
    mask_rr = (dists <= RR) & ~eye_b
    mask_ro = (dists <= RO + RC) | eye_b

    def wrap(diff):
        diff = np.where(diff <= -PI, np.mod(diff, np.float32(PI)), diff)
        return diff - (diff >= PI).astype(np.float32) * np.float32(2.0 * PI)

    ang = np.angle(ori).astype(np.float32)
    abs_ad = np.abs(wrap(ang[:, None] - ang[None, :]))
    in_front = abs_ad < PI / 2

    Wrr = (mask_rr & in_front).astype(np.float32)
    Wro = mask_ro.astype(np.float32)

    def mvc(W, z):
        return (W @ z.real.astype(np.float32)
                + 1j * (W @ z.imag.astype(np.float32))).astype(np.complex64)

    n_r = Wrr.sum(axis=1)
    S = mvc(Wrr, pos) / np.maximum(n_r, 1.0) - pos * np.sign(n_r)
    d = -S
    cms = pos.sum() / np.float32(n)
    Ps = (cms - pos).astype(np.complex64)
    osum = mvc(Wro, ori)
    left = Ps * np.exp(1j * deltas).astype(np.complex64)
    right = Ps * np.exp(-1j * deltas).astype(np.complex64)

    def cossim(a, b):
        dot = a.real * b.real + a.imag * b.imag
        na = np.maximum(np.abs(a), 1e-14)
        nb = np.maximum(np.abs(b), 1e-14)
        return dot / (na * nb)

    best = np.where(cossim(left, osum) >= cossim(right, osum), left, right)
    has_rep = np.abs(d) > 0.0
    z1 = np.where(has_rep, d, best)
    att = wrap(np.angle(z1).astype(np.float32) - np.angle(ori).astype(np.float32))
    s2dr = np.float32(np.sqrt(2.0 * DR))
    sdt = np.float32(np.sqrt(DT))
    rot = np.exp(1j * (DT * GAMMA * DR * np.sin(att)
                       + rot_noise * s2dr * sdt)).astype(np.complex64)
    tnoise = ((tn_re + 1j * tn_im) * np.float32(np.sqrt(0.5))
              * np.float32(np.sqrt(2.0 * DT_TRANS))).astype(np.complex64)
    trans = (np.float32(DT * VEL) * ori + tnoise * sdt).astype(np.complex64)

    p = (pos + trans).astype(np.complex64)
    cont = True
    for _ in range(COLL_ITERS):
        diff = p[None, :] - p[:, None] + eye_c
        absd = np.abs(diff).astype(np.float32)
        coll = absd <= (2.0 * RC - eye_f)
        move = np.where(coll,
                        diff / np.where(coll, absd, 1.0)
                        * ((2.1 * RC - absd) * 0.5), 0.0)
        if cont:
            p = (p - move.sum(axis=1)).astype(np.complex64)
            cont = bool(coll.sum() > 0)
    new_pos = p
    new_ori = (ori * rot).astype(np.complex64)

    def c2r(z):
        return np.stack([z.real, z.imag], axis=-1).astype(np.float32)

    return np.stack([c2r(new_pos), c2r(new_ori), c2r(osum), c2r(left),
                     c2r(right)], axis=0)


def _asm(res, key):
    """[128, NT, 2] per-core outputs -> [N, 2] sorted order."""
    out = np.empty((N, 2), np.float32)
    for c in range(NCORES):
        a = res[c][key]  # [128, NT, 2]
        out[c * ROWS:(c + 1) * ROWS] = a.transpose(1, 0, 2).reshape(ROWS, 2)
    return out


def kernel(pos_re, pos_im, ori_re, ori_im, deltas, rot_noise, tn_re, tn_im):
    global _BUILT, LAST_EXEC_NS
    import sys
    sys.path.insert(0, "/opt/trn_rl_repo")
    from concourse.bass_utils import run_bass_kernel_spmd

    ins = dict(pos_re=pos_re, pos_im=pos_im, ori_re=ori_re, ori_im=ori_im,
               deltas=deltas, rot_noise=rot_noise, tn_re=tn_re, tn_im=tn_im)
    ins = {k: np.asarray(v, np.float32) for k, v in ins.items()}

    # host: trans + sort permutation (schedule only; guarded below)
    trans_x = C_DV * ins["ori_re"] + C_TN * ins["tn_re"]
    trans_y = C_DV * ins["ori_im"] + C_TN * ins["tn_im"]
    p0x = ins["pos_re"] + trans_x
    p0y = ins["pos_im"] + trans_y
    perm = np.argsort(p0x, kind="stable")
    ins_s = {k: v[perm] for k, v in ins.items()}
    p0x_s, p0y_s = p0x[perm], p0y[perm]

    # ---- host-side band guards (exact) ----
    xs = ins_s["pos_re"].astype(np.float64)
    premax = np.maximum.accumulate(xs)
    sufmin = np.minimum.accumulate(xs[::-1])[::-1]
    ok_f = bool(np.all(sufmin[BF:] - premax[:-BF] > (RO + RC) * 1.001))
    gap128 = float(np.min(p0x_s[BC:].astype(np.float64)
                          - p0x_s[:-BC].astype(np.float64)))

    if not ok_f:
        out_full = _reference_numpy(**ins)
        return out_full

    if _BUILT is None:
        _BUILT = _build_nc()
    nc = _BUILT

    in_maps = _prepare_inputs(ins_s, p0x_s, p0y_s)
    r = run_bass_kernel_spmd(nc, in_maps, core_ids=list(range(NCORES)),
                             trace=TRACE)
    LAST_EXEC_NS = r.exec_time_ns
    res = r.results

    # collision guard: max displacement vs sorted-gap bound
    dmax = 0.0
    for c in range(NCORES):
        g = res[c]["o_guard"].astype(np.float64)
        dmax = max(dmax, float(np.max(np.sqrt(g[:, :, 0] + g[:, :, 1]))))
    gap_wrap = float(p0x_s[3584].astype(np.float64) - p0x_s[511].astype(np.float64))
    if not (gap128 > 2.0 * RC * 1.001 + 2.0 * dmax + 1e-8
            and gap_wrap > 2.0 * RC * 1.001 + 2.0 * dmax + 1e-8):
        return _reference_numpy(**ins)

    out_s = np.stack([_asm(res, "o_pos"), _asm(res, "o_ori"),
                      _asm(res, "o_osum"), _asm(res, "o_left"),
                      _asm(res, "o_right")], axis=0)
    out = np.empty_like(out_s)
    out[:, perm, :] = out_s
    return out


# revision 30
# speedup vs baseline: 1.6629x; 1.0483x over previous
"""Trainium2 Bass kernel for the ActiveParticles nn.Module (N=4096, 8 cores).

Strategy
--------
Host sorts particles by post-translation x coordinate (a pure permutation —
the reference computation is permutation-equivariant). In sorted space every
interaction radius becomes a *rank band*:
  - collision radius 2*RC  -> band +-128 ranks   (30-iteration loop)
  - orientation radius RO+RC -> band +-384 ranks (one-time masks)
Each of the 8 cores owns 512 consecutive sorted rows and computes only a
[128 x band] tile strip per row-tile. The collision loop allgathers the
updated 512x2 positions every iteration. Band validity is *proved* per input:
host-side exact gap checks + a device-computed max-displacement guard; if any
check fails, a dense numpy fallback (exact reference semantics) is used.

Outputs are produced in sorted space and unpermuted on host.
"""

import os
import numpy as np

# ---------------------------------------------------------------- constants
PI = 3.1415927410125732
N = 4096
L = 1.28e-3
VEL = 5e-7
RC = 3.15e-6
RR = 8e-6
RO = 2.5e-5
DT_TRANS = 1.4e-14
DR = 0.0028
GAMMA = 25.0
DT = 0.2
COLL_ITERS = 30

NCORES = 8
ROWS = N // NCORES          # 512 rows per core
NT = ROWS // 128            # 4 row-tiles per core

BC = 128                    # collision band (ranks each side)
CW = 2 * BC + 128           # collision tile window = 384
CWIN = ROWS + 2 * BC        # per-core collision window = 768

BF = 384                    # forward (mask) band
FW = 2 * BF + 128           # forward tile window = 896
FWIN = ROWS + 2 * BF        # per-core forward window = 1280

PAD = 128                   # sentinel pad rows each side of the shared array
NPAD = N + 2 * PAD          # 4352
SENT = np.float32(1.0e9)

F32 = np.float32
C2 = F32((2.0 * RC) ** 2)          # collision threshold on squared dist
FAC = F32(1.05 * RC)               # (2.1*RC)*0.5
RR2 = F32(RR ** 2)
RO2 = F32((RO + RC) ** 2)
C_DV = F32(DT * VEL)
C_TN = F32(np.sqrt(0.5) * np.sqrt(2.0 * DT_TRANS) * np.sqrt(DT))
C_GGD = F32(DT * GAMMA * DR)
C_RN = F32(np.sqrt(2.0 * DR) * np.sqrt(DT))
HALF_PI = F32(PI / 2)

_BUILT = None
TRACE = False               # test.py sets True to collect exec_time_ns
LAST_EXEC_NS = None


# =====================================================================
# Device program
# =====================================================================
def _build_nc():
    import concourse.bass as bass
    import concourse.mybir as mybir
    from concourse import tile, bass_isa, library_config
    from concourse import bacc

    f32 = mybir.dt.float32
    A = mybir.AluOpType
    AF = mybir.ActivationFunctionType

    nc = bacc.Bacc("TRN2", target_bir_lowering=False, debug=False,
                   num_devices=NCORES)

    # ---------------- I/O ----------------
    t_own = nc.dram_tensor("own8", [128, NT, 8], f32, kind="ExternalInput")
    t_cwin = nc.dram_tensor("cwin", [6, CWIN], f32, kind="ExternalInput")
    t_fwin = nc.dram_tensor("fwin", [4, FWIN], f32, kind="ExternalInput")
    t_fpx = nc.dram_tensor("fpx", [1, N], f32, kind="ExternalInput")
    t_fpy = nc.dram_tensor("fpy", [1, N], f32, kind="ExternalInput")
    t_eye = nc.dram_tensor("eye128", [128, 128], f32, kind="ExternalInput")

    o_pos = nc.dram_tensor("o_pos", [128, NT, 2], f32, kind="ExternalOutput")
    o_ori = nc.dram_tensor("o_ori", [128, NT, 2], f32, kind="ExternalOutput")
    o_osum = nc.dram_tensor("o_osum", [128, NT, 2], f32, kind="ExternalOutput")
    o_left = nc.dram_tensor("o_left", [128, NT, 2], f32, kind="ExternalOutput")
    o_right = nc.dram_tensor("o_right", [128, NT, 2], f32, kind="ExternalOutput")
    o_guard = nc.dram_tensor("o_guard", [128, NT, 2], f32, kind="ExternalOutput")

    rg = [list(range(NCORES))]

    with tile.TileContext(nc) as tc:
        dve, act, gps, dma = nc.vector, nc.scalar, nc.gpsimd, nc.sync
        with tc.tile_pool(name="main", bufs=1) as mp, \
             tc.tile_pool(name="dram", bufs=2, space="DRAM") as dp:

            def T(shape, name, dt=f32):
                return mp.tile(shape, dt, name=name, tag=name, bufs=1)

            # persistent SBUF state
            own = T([128, NT, 8], "own")
            cwx = T([1, CWIN], "cwx")
            cwy = T([1, CWIN], "cwy")
            cwo1 = T([1, CWIN], "cwo1")
            cwo2 = T([1, CWIN], "cwo2")
            cwt1 = T([1, CWIN], "cwt1")
            cwt2 = T([1, CWIN], "cwt2")
            fwx = T([1, FWIN], "fwx")
            fwy = T([1, FWIN], "fwy")
            fwo1 = T([1, FWIN], "fwo1")
            fwo2 = T([1, FWIN], "fwo2")
            eye = T([128, 128], "eye")
            fpx = T([1, N], "fpx")
            fpy = T([1, N], "fpy")
            px = T([128, NT], "px")
            py = T([128, NT], "py")
            px0 = T([128, NT], "px0")
            py0 = T([128, NT], "py0")
            gx = T([128, NT], "gx")
            gy = T([128, NT], "gy")
            sumx = T([128, NT], "sumx")
            sumy = T([128, NT], "sumy")
            nrt_ = T([128, NT], "nrt")
            sxt = T([128, NT], "sxt")
            syt = T([128, NT], "syt")
            oxt = T([128, NT], "oxt")
            oyt = T([128, NT], "oyt")
            cbias = T([128, 1], "cbias")
            cbias5 = T([128, 1], "cbias5")
            npxn = T([128, NT], "npxn")
            npyn = T([128, NT], "npyn")

            dma.dma_start(own[:], t_own[:])
            dma.dma_start(cwx[:], t_cwin[0:1, :])
            dma.dma_start(cwy[:], t_cwin[1:2, :])
            dma.dma_start(cwo1[:], t_cwin[2:3, :])
            dma.dma_start(cwo2[:], t_cwin[3:4, :])
            dma.dma_start(cwt1[:], t_cwin[4:5, :])
            dma.dma_start(cwt2[:], t_cwin[5:6, :])
            dma.dma_start(fwx[:], t_fwin[0:1, :])
            dma.dma_start(fwy[:], t_fwin[1:2, :])
            dma.dma_start(fwo1[:], t_fwin[2:3, :])
            dma.dma_start(fwo2[:], t_fwin[3:4, :])
            dma.dma_start(eye[:], t_eye[:])
            dma.dma_start(fpx[:], t_fpx[:])
            dma.dma_start(fpy[:], t_fpy[:])
            dve.memset(cbias[:], float(HALF_PI))
            dve.memset(cbias5[:], -0.5)


            def ow(k):
                return own[:, :, k]

            # seed p0 = pos + trans (own rows)
            dve.scalar_tensor_tensor(px[:], ow(2), float(C_DV), ow(0), A.mult, A.add)
            dve.scalar_tensor_tensor(px[:], ow(6), float(C_TN), px[:], A.mult, A.add)
            dve.scalar_tensor_tensor(py[:], ow(3), float(C_DV), ow(1), A.mult, A.add)
            dve.scalar_tensor_tensor(py[:], ow(7), float(C_TN), py[:], A.mult, A.add)
            dve.tensor_copy(px0[:], px[:])
            dve.tensor_copy(py0[:], py[:])
            dve.memset(gx[:], 0.0)
            dve.memset(gy[:], 0.0)

            # seed window p0 = pos + trans
            dve.scalar_tensor_tensor(cwx[:], cwo1[:], float(C_DV), cwx[:],
                                     A.mult, A.add)
            dve.scalar_tensor_tensor(cwx[:], cwt1[:], float(C_TN), cwx[:],
                                     A.mult, A.add)
            dve.scalar_tensor_tensor(cwy[:], cwo2[:], float(C_DV), cwy[:],
                                     A.mult, A.add)
            dve.scalar_tensor_tensor(cwy[:], cwt2[:], float(C_TN), cwy[:],
                                     A.mult, A.add)

            # =====================================================
            # FORWARD one-time: masks + reductions (band BF)
            # =====================================================
            with tc.tile_pool(name="fsb", bufs=2) as fp:
                pxb = fp.tile([128, FWIN], f32, name="pxb", tag="pxb", bufs=1)
                pyb = fp.tile([128, FWIN], f32, name="pyb", tag="pyb", bufs=1)
                oreb = fp.tile([128, FWIN], f32, name="oreb", tag="oreb", bufs=1)
                oimb = fp.tile([128, FWIN], f32, name="oimb", tag="oimb", bufs=1)
                gps.partition_broadcast(pxb[:], fwx[:])
                gps.partition_broadcast(pyb[:], fwy[:])
                gps.partition_broadcast(oreb[:], fwo1[:])
                gps.partition_broadcast(oimb[:], fwo2[:])

                for t in range(NT):
                    w0 = 128 * t
                    pxi = own[:, t, 0:1]
                    pyi = own[:, t, 1:2]
                    orei = own[:, t, 2:3]
                    oimi = own[:, t, 3:4]
                    fdx = fp.tile([128, FW], f32, tag="fdx")
                    fdy = fp.tile([128, FW], f32, tag="fdy")
                    ft1 = fp.tile([128, FW], f32, tag="ft1")
                    ft2 = fp.tile([128, FW], f32, tag="ft2")
                    fsq = fp.tile([128, FW], f32, tag="fsq")
                    fg = fp.tile([128, FW], f32, tag="fg")
                    fm1 = fp.tile([128, FW], f32, tag="fm1")
                    fm2 = fp.tile([128, FW], f32, tag="fm2")
                    fwrr = fp.tile([128, FW], f32, tag="fwrr")
                    fwro = fp.tile([128, FW], f32, tag="fwro")

                    dve.tensor_scalar(fdx[:], pxb[:, w0:w0 + FW], pxi, None,
                                      A.subtract)
                    dve.tensor_scalar(fdy[:], pyb[:, w0:w0 + FW], pyi, None,
                                      A.subtract)
                    dve.tensor_tensor(fdx[:, BF:BF + 128], fdx[:, BF:BF + 128],
                                      eye[:], A.add)
                    act.activation(ft1[:], fdx[:], AF.Square)
                    act.activation(ft2[:], fdy[:], AF.Square)
                    dve.tensor_tensor(fsq[:], ft1[:], ft2[:], A.add)
                    dve.tensor_scalar(fg[:], oimb[:, w0:w0 + FW], oimi, None,
                                      A.mult)
                    dve.scalar_tensor_tensor(fg[:], oreb[:, w0:w0 + FW], orei,
                                             fg[:], A.mult, A.add)
                    dve.tensor_scalar(fm1[:], fsq[:], float(RR2), None, A.is_le)
                    dve.tensor_scalar(fm2[:], fg[:], 0.0, None, A.is_gt)
                    dve.tensor_tensor(fwrr[:], fm1[:], fm2[:], A.mult)
                    dve.tensor_scalar(fwro[:], fsq[:], float(RO2), None, A.is_le)
                    dve.tensor_tensor(fwro[:, BF:BF + 128],
                                      fwro[:, BF:BF + 128], eye[:], A.max)

                    dve.tensor_scalar(fwrr[:], fwrr[:], 1.0, None, A.mult,
                                      A.add, accum_out=nrt_[:, t:t + 1])
                    dve.scalar_tensor_tensor(ft1[:], fwrr[:], 1.0,
                                             pxb[:, w0:w0 + FW],
                                             A.mult, A.mult,
                                             accum_out=sxt[:, t:t + 1])
                    dve.scalar_tensor_tensor(ft2[:], fwrr[:], 1.0,
                                             pyb[:, w0:w0 + FW],
                                             A.mult, A.mult,
                                             accum_out=syt[:, t:t + 1])
                    dve.scalar_tensor_tensor(fdx[:], fwro[:], 1.0,
                                             oreb[:, w0:w0 + FW],
                                             A.mult, A.mult,
                                             accum_out=oxt[:, t:t + 1])
                    dve.scalar_tensor_tensor(fdy[:], fwro[:], 1.0,
                                             oimb[:, w0:w0 + FW],
                                             A.mult, A.mult,
                                             accum_out=oyt[:, t:t + 1])

            # ---------------- per-particle forward math [128, NT] --------
            def tt(name):
                return T([128, NT], name)

            c1x = T([1, 16], "c1x")
            c1y = T([1, 16], "c1y")
            cmsxb16 = T([128, 16], "cmsxb16")
            cmsyb16 = T([128, 16], "cmsyb16")
            dve.memset(c1x[:], 0.0)
            dve.memset(c1y[:], 0.0)
            dve.tensor_reduce(c1x[:, 0:1], fpx[:], mybir.AxisListType.X, A.add)
            dve.tensor_reduce(c1y[:, 0:1], fpy[:], mybir.AxisListType.X, A.add)
            dve.tensor_scalar(c1x[:], c1x[:], float(F32(1.0 / N)), None, A.mult)
            dve.tensor_scalar(c1y[:], c1y[:], float(F32(1.0 / N)), None, A.mult)
            gps.partition_broadcast(cmsxb16[:], c1x[:])
            gps.partition_broadcast(cmsyb16[:], c1y[:])
            cmsxb = cmsxb16[:, 0:1]
            cmsyb = cmsyb16[:, 0:1]

            nrx = tt("nrx")
            inr = tt("inr")
            snr = tt("snr")
            dve.tensor_scalar(nrx[:], nrt_[:], 1.0, None, A.max)
            dve.reciprocal(inr[:], nrx[:])
            dve.tensor_scalar(snr[:], nrt_[:], 0.0, None, A.is_gt)
            dxv = tt("dxv")
            dyv = tt("dyv")
            w1 = tt("w1")
            w2 = tt("w2")
            dve.tensor_tensor(w1[:], sxt[:], inr[:], A.mult)
            dve.tensor_tensor(w2[:], ow(0), snr[:], A.mult)
            dve.tensor_tensor(dxv[:], w2[:], w1[:], A.subtract)
            dve.tensor_tensor(w1[:], syt[:], inr[:], A.mult)
            dve.tensor_tensor(w2[:], ow(1), snr[:], A.mult)
            dve.tensor_tensor(dyv[:], w2[:], w1[:], A.subtract)

            psx = tt("psx")
            psy = tt("psy")
            dve.tensor_scalar(psx[:], ow(0), cmsxb, -1.0,
                              A.subtract, A.mult)
            dve.tensor_scalar(psy[:], ow(1), cmsyb, -1.0,
                              A.subtract, A.mult)

            cd = tt("cd")
            sd = tt("sd")
            act.activation(cd[:], ow(4), AF.Sin, bias=cbias[:])
            act.activation(sd[:], ow(4), AF.Sin)
            ta = tt("ta")
            tb = tt("tb")
            tc_ = tt("tc_")
            td = tt("td")
            dve.tensor_tensor(ta[:], psx[:], cd[:], A.mult)
            dve.tensor_tensor(tb[:], psy[:], sd[:], A.mult)
            dve.tensor_tensor(tc_[:], psy[:], cd[:], A.mult)
            dve.tensor_tensor(td[:], psx[:], sd[:], A.mult)
            lx = tt("lx")
            ly = tt("ly")
            rx = tt("rx")
            ry = tt("ry")
            dve.tensor_tensor(lx[:], ta[:], tb[:], A.subtract)
            dve.tensor_tensor(ly[:], tc_[:], td[:], A.add)
            dve.tensor_tensor(rx[:], ta[:], tb[:], A.add)
            dve.tensor_tensor(ry[:], tc_[:], td[:], A.subtract)

            dl = tt("dl")
            dr_ = tt("dr_")
            dve.tensor_tensor(w1[:], lx[:], oxt[:], A.mult)
            dve.tensor_tensor(w2[:], ly[:], oyt[:], A.mult)
            dve.tensor_tensor(dl[:], w1[:], w2[:], A.add)
            dve.tensor_tensor(w1[:], rx[:], oxt[:], A.mult)
            dve.tensor_tensor(w2[:], ry[:], oyt[:], A.mult)
            dve.tensor_tensor(dr_[:], w1[:], w2[:], A.add)
            mlr = tt("mlr")
            mlrc = tt("mlrc")
            dve.tensor_tensor(w1[:], dl[:], dr_[:], A.subtract)
            dve.tensor_scalar(mlr[:], w1[:], 0.0, None, A.is_ge)
            dve.tensor_scalar(mlrc[:], mlr[:], -1.0, 1.0, A.mult, A.add)
            bx = tt("bx")
            by = tt("by")
            dve.tensor_tensor(w1[:], mlr[:], lx[:], A.mult)
            dve.tensor_tensor(w2[:], mlrc[:], rx[:], A.mult)
            dve.tensor_tensor(bx[:], w1[:], w2[:], A.add)
            dve.tensor_tensor(w1[:], mlr[:], ly[:], A.mult)
            dve.tensor_tensor(w2[:], mlrc[:], ry[:], A.mult)
            dve.tensor_tensor(by[:], w1[:], w2[:], A.add)

            hr = tt("hr")
            dve.tensor_tensor(w1[:], dxv[:], dxv[:], A.mult)
            dve.tensor_tensor(w2[:], dyv[:], dyv[:], A.mult)
            dve.tensor_tensor(w1[:], w1[:], w2[:], A.add)
            dve.tensor_scalar(hr[:], w1[:], 0.0, None, A.is_gt)
            hrc = tt("hrc")
            dve.tensor_scalar(hrc[:], hr[:], -1.0, 1.0, A.mult, A.add)
            zx = tt("zx")
            zy = tt("zy")
            dve.tensor_tensor(w1[:], hr[:], dxv[:], A.mult)
            dve.tensor_tensor(w2[:], hrc[:], bx[:], A.mult)
            dve.tensor_tensor(zx[:], w1[:], w2[:], A.add)
            dve.tensor_tensor(w1[:], hr[:], dyv[:], A.mult)
            dve.tensor_tensor(w2[:], hrc[:], by[:], A.mult)
            dve.tensor_tensor(zy[:], w1[:], w2[:], A.add)
            wx = tt("wx")
            wy = tt("wy")
            dve.tensor_tensor(w1[:], zx[:], ow(2), A.mult)
            dve.tensor_tensor(w2[:], zy[:], ow(3), A.mult)
            dve.tensor_tensor(wx[:], w1[:], w2[:], A.add)
            dve.tensor_tensor(w1[:], zy[:], ow(2), A.mult)
            dve.tensor_tensor(w2[:], zx[:], ow(3), A.mult)
            dve.tensor_tensor(wy[:], w1[:], w2[:], A.subtract)
            # sin(att) = sin(angle(z1) - angle(ori)) = wy / |w|
            satt = tt("satt")
            act.activation(w1[:], wx[:], AF.Square)
            act.activation(w2[:], wy[:], AF.Square)
            dve.tensor_tensor(w1[:], w1[:], w2[:], A.add)
            dve.tensor_scalar(w1[:], w1[:], 1.0e-37, None, A.max)
            dve.reciprocal(w1[:], w1[:])
            act.activation(w1[:], w1[:], AF.Sqrt)
            dve.tensor_tensor(satt[:], wy[:], w1[:], A.mult)
            th = tt("th")
            dve.tensor_scalar(w1[:], satt[:], float(C_GGD), None, A.mult)
            dve.scalar_tensor_tensor(th[:], ow(5), float(C_RN), w1[:], A.mult, A.add)
            cth = tt("cth")
            sth = tt("sth")
            act.activation(cth[:], th[:], AF.Sin, bias=cbias[:])
            act.activation(sth[:], th[:], AF.Sin)
            nox = tt("nox")
            noy = tt("noy")
            dve.tensor_tensor(w1[:], ow(2), cth[:], A.mult)
            dve.tensor_tensor(w2[:], ow(3), sth[:], A.mult)
            dve.tensor_tensor(nox[:], w1[:], w2[:], A.subtract)
            dve.tensor_tensor(w1[:], ow(3), cth[:], A.mult)
            dve.tensor_tensor(w2[:], ow(2), sth[:], A.mult)
            dve.tensor_tensor(noy[:], w1[:], w2[:], A.add)

            dma.dma_start(o_ori[:, :, 0], nox[:])
            dma.dma_start(o_ori[:, :, 1], noy[:])
            dma.dma_start(o_osum[:, :, 0], oxt[:])
            dma.dma_start(o_osum[:, :, 1], oyt[:])
            dma.dma_start(o_left[:, :, 0], lx[:])
            dma.dma_start(o_left[:, :, 1], ly[:])
            dma.dma_start(o_right[:, :, 0], rx[:])
            dma.dma_start(o_right[:, :, 1], ry[:])

            # =====================================================
            # COLLISION LOOP (30 iterations, band BC)
            # =====================================================
            cpxb = T([128, CWIN], "cpxb")
            cpyb = T([128, CWIN], "cpyb")
            npx = T([128, NT], "npx")
            npy = T([128, NT], "npy")
            tmpd = T([128, NT], "tmpd")

            with tc.tile_pool(name="csb", bufs=3) as cp:
                fx = cp.tile([1, NPAD], f32, name="fx", tag="fx", bufs=1)
                fy = cp.tile([1, NPAD], f32, name="fy", tag="fy", bufs=1)
                dve.memset(fx[:], float(SENT))
                dve.memset(fy[:], float(SENT))
                prev_agout = None
                woff_h = []
                for it in range(COLL_ITERS):
                    if it > 0:
                        if not woff_h:
                            woff_h.append(dve.snap(
                                dve.partition_id() * ROWS, donate=False))
                        woff = woff_h[0]
                        agx = prev_agout[:].rearrange("(r two) w -> two r w",
                                                      two=2)
                        dma.dma_start(fx[0:1, PAD:PAD + N], agx[0])
                        dma.dma_start(fy[0:1, PAD:PAD + N], agx[1])
                        dve.tensor_copy(cwx[0:1, 0:CWIN],
                                        fx[0:1, bass.ds(woff, CWIN)])
                        dve.tensor_copy(cwy[0:1, 0:CWIN],
                                        fy[0:1, bass.ds(woff, CWIN)])
                    gps.partition_broadcast(cpxb[:], cwx[:])
                    gps.partition_broadcast(cpyb[:], cwy[:])

                    cdx = cp.tile([128, NT, CW], f32, tag="cdx", bufs=1)
                    cdy = cp.tile([128, NT, CW], f32, tag="cdy", bufs=1)
                    ct1 = cp.tile([128, NT, CW], f32, tag="ct1", bufs=1)
                    ct2 = cp.tile([128, NT, CW], f32, tag="ct2", bufs=1)
                    csq = cp.tile([128, NT, CW], f32, tag="csq", bufs=1)
                    cfm = cp.tile([128, NT, CW], f32, tag="cfm", bufs=1)
                    cmx = cp.tile([128, NT, CW], f32, tag="cmx", bufs=1)
                    act.mul(npxn[:], px[:], -1.0)
                    act.mul(npyn[:], py[:], -1.0)
                    for t in range(NT):
                        w0 = 128 * t
                        act.activation(cdx[:, t, :], cpxb[:, w0:w0 + CW],
                                       AF.Identity, bias=npxn[:, t:t + 1])
                        act.activation(cdy[:, t, :], cpyb[:, w0:w0 + CW],
                                       AF.Identity, bias=npyn[:, t:t + 1])
                        dve.tensor_tensor(cdx[:, t, BC:BC + 128],
                                          cdx[:, t, BC:BC + 128], eye[:], A.add)
                    act.activation(ct1[:], cdx[:], AF.Square)
                    act.activation(ct2[:], cdy[:], AF.Square)
                    dve.tensor_tensor(csq[:], ct1[:], ct2[:], A.add)
                    dve.reciprocal(ct1[:], csq[:])
                    # FAC/sqrt(csq) - 0.5 = Sqrt(FAC^2 / csq) - 0.5
                    act.activation(ct2[:], ct1[:], AF.Sqrt,
                                   scale=float(F32(FAC * FAC)))
                    act.activation(ct2[:], ct2[:], AF.Identity, bias=cbias5[:])
                    dve.scalar_tensor_tensor(cfm[:], csq[:], float(C2), ct2[:],
                                             A.is_le, A.mult)
                    for t in range(NT):
                        dve.scalar_tensor_tensor(
                            cmx[:, t, :], cdx[:, t, :], 1.0, cfm[:, t, :],
                            A.mult, A.mult, accum_out=sumx[:, t:t + 1])
                        dve.scalar_tensor_tensor(
                            cmx[:, t, :], cdy[:, t, :], 1.0, cfm[:, t, :],
                            A.mult, A.mult, accum_out=sumy[:, t:t + 1])

                    dve.tensor_tensor(px[:], px[:], sumx[:], A.subtract)
                    dve.tensor_tensor(py[:], py[:], sumy[:], A.subtract)

                    dve.tensor_tensor(tmpd[:], px[:], px0[:], A.subtract)
                    dve.tensor_tensor(tmpd[:], tmpd[:], tmpd[:], A.mult)
                    dve.tensor_tensor(gx[:], gx[:], tmpd[:], A.max)
                    dve.tensor_tensor(tmpd[:], py[:], py0[:], A.subtract)
                    dve.tensor_tensor(tmpd[:], tmpd[:], tmpd[:], A.mult)
                    dve.tensor_tensor(gy[:], gy[:], tmpd[:], A.max)

                    if it < COLL_ITERS - 1:
                        agin = dp.tile([2, ROWS], f32, name="agin",
                                       tag="agin", bufs=2)
                        agout = dp.tile([2 * NCORES, ROWS], f32, name="agout",
                                        tag="agout", bufs=2,
                                        addr_space="Shared")
                        agr = agin[:].rearrange("c (t p) -> c p t", p=128)
                        dma.dma_start(agr[0], px[:])
                        dma.dma_start(agr[1], py[:])
                        gps.collective_compute(
                            "AllGather", A.bypass, replica_groups=rg,
                            ins=[agin[:].opt()],
                            outs=[agout[:].opt()])
                        prev_agout = agout

            dma.dma_start(o_pos[:, :, 0], px[:])
            dma.dma_start(o_pos[:, :, 1], py[:])
            dma.dma_start(o_guard[:, :, 0], gx[:])
            dma.dma_start(o_guard[:, :, 1], gy[:])

    nc.compile()
    return nc


# =====================================================================
# Host-side helpers
# =====================================================================
def _win(arr_s, c, band, fill):
    """arr_s padded window [c*ROWS - band, c*ROWS + ROWS + band)."""
    lo = c * ROWS - band
    hi = c * ROWS + ROWS + band
    out = np.full(hi - lo, fill, dtype=np.float32)
    a, b = max(lo, 0), min(hi, N)
    out[a - lo:b - lo] = arr_s[a:b]
    return out


def _prepare_inputs(ins_s, p0x_s, p0y_s):
    """Per-core input dicts from sorted arrays."""
    names = ["pos_re", "pos_im", "ori_re", "ori_im", "deltas", "rot_noise",
             "tn_re", "tn_im"]
    eye = np.eye(128, dtype=np.float32)
    fpx = ins_s["pos_re"][None, :].copy()
    fpy = ins_s["pos_im"][None, :].copy()
    in_maps = []
    for c in range(NCORES):
        sl = slice(c * ROWS, (c + 1) * ROWS)
        own = np.stack([ins_s[k][sl] for k in names], axis=-1)  # [512, 8]
        own = own.reshape(NT, 128, 8).transpose(1, 0, 2).copy()  # [128,4,8]

        cwin = np.stack([
            _win(ins_s["pos_re"], c, BC, SENT),
            _win(ins_s["pos_im"], c, BC, SENT),
            _win(ins_s["ori_re"], c, BC, 0.0),
            _win(ins_s["ori_im"], c, BC, 0.0),
            _win(ins_s["tn_re"], c, BC, 0.0),
            _win(ins_s["tn_im"], c, BC, 0.0),
        ])
        fwin = np.stack([
            _win(ins_s["pos_re"], c, BF, SENT),
            _win(ins_s["pos_im"], c, BF, SENT),
            _win(ins_s["ori_re"], c, BF, 0.0),
            _win(ins_s["ori_im"], c, BF, 0.0),
        ])
        in_maps.append({
            "own8": own, "cwin": cwin, "fwin": fwin, "fpx": fpx, "fpy": fpy,
            "eye128": eye,
        })
    return in_maps


def _reference_numpy(pos_re, pos_im, ori_re, ori_im, deltas, rot_noise,
                     tn_re, tn_im):
    """Dense fp32 fallback with exact reference semantics."""
    pos = (pos_re + 1j * pos_im).astype(np.complex64)
    ori = (ori_re + 1j * ori_im).astype(np.complex64)
    n = pos.shape[0]
    eye_b = np.eye(n, dtype=bool)
    eye_f = np.eye(n, dtype=np.float32)
    eye_c = eye_f.astype(np.complex64)

    dists = np.abs(pos[:, None] - pos[None, :] + eye_c).astype(np.float32)
    mask_rr = (dists <= RR) & ~eye_b
    mask_ro = (dists <= RO + RC) | eye_b

    def wrap(diff):
        diff = np.where(diff <= -PI, np.mod(diff, np.float32(PI)), diff)
        return diff - (diff >= PI).astype(np.float32) * np.float32(2.0 * PI)

    ang = np.angle(ori).astype(np.float32)
    abs_ad = np.abs(wrap(ang[:, None] - ang[None, :]))
    in_front = abs_ad < PI / 2

    Wrr = (mask_rr & in_front).astype(np.float32)
    Wro = mask_ro.astype(np.float32)

    def mvc(W, z):
        return (W @ z.real.astype(np.float32)
                + 1j * (W @ z.imag.astype(np.float32))).astype(np.complex64)

    n_r = Wrr.sum(axis=1)
    S = mvc(Wrr, pos) / np.maximum(n_r, 1.0) - pos * np.sign(n_r)
    d = -S
    cms = pos.sum() / np.float32(n)
    Ps = (cms - pos).astype(np.complex64)
    osum = mvc(Wro, ori)
    left = Ps * np.exp(1j * deltas).astype(np.complex64)
    right = Ps * np.exp(-1j * deltas).astype(np.complex64)

    def cossim(a, b):
        dot = a.real * b.real + a.imag * b.imag
        na = np.maximum(np.abs(a), 1e-14)
        nb = np.maximum(np.abs(b), 1e-14)
        return dot / (na * nb)

    best = np.where(cossim(left, osum) >= cossim(right, osum), left, right)
    has_rep = np.abs(d) > 0.0
    z1 = np.where(has_rep, d, best)
    att = wrap(np.angle(z1).astype(np.float32) - np.angle(ori).astype(np.float32))
    s2dr = np.float32(np.sqrt(2.0 * DR))
    sdt = np.float32(np.sqrt(DT))
    rot = np.exp(1j * (DT * GAMMA * DR * np.sin(att)
                       + rot_noise * s2dr * sdt)).astype(np.complex64)
    tnoise = ((tn_re + 1j * tn_im) * np.float32(np.sqrt(0.5))
              * np.float32(np.sqrt(2.0 * DT_TRANS))).astype(np.complex64)
    trans = (np.float32(DT * VEL) * ori + tnoise * sdt).astype(np.complex64)

    p = (pos + trans).astype(np.complex64)
    cont = True
    for _ in range(COLL_ITERS):
        diff = p[None, :] - p[:, None] + eye_c
        absd = np.abs(diff).astype(np.float32)
        coll = absd <= (2.0 * RC - eye_f)
        move = np.where(coll,
                        diff / np.where(coll, absd, 1.0)
                        * ((2.1 * RC - absd) * 0.5), 0.0)
        if cont:
            p = (p - move.sum(axis=1)).astype(np.complex64)
            cont = bool(coll.sum() > 0)
    new_pos = p
    new_ori = (ori * rot).astype(np.complex64)

    def c2r(z):
        return np.stack([z.real, z.imag], axis=-1).astype(np.float32)

    return np.stack([c2r(new_pos), c2r(new_ori), c2r(osum), c2r(left),
                     c2r(right)], axis=0)


def _asm(res, key):
    """[128, NT, 2] per-core outputs -> [N, 2] sorted order."""
    out = np.empty((N, 2), np.float32)
    for c in range(NCORES):
        a = res[c][key]  # [128, NT, 2]
        out[c * ROWS:(c + 1) * ROWS] = a.transpose(1, 0, 2).reshape(ROWS, 2)
    return out


def kernel(pos_re, pos_im, ori_re, ori_im, deltas, rot_noise, tn_re, tn_im):
    global _BUILT, LAST_EXEC_NS
    import sys
    sys.path.insert(0, "/opt/trn_rl_repo")
    from concourse.bass_utils import run_bass_kernel_spmd

    ins = dict(pos_re=pos_re, pos_im=pos_im, ori_re=ori_re, ori_im=ori_im,
               deltas=deltas, rot_noise=rot_noise, tn_re=tn_re, tn_im=tn_im)
    ins = {k: np.asarray(v, np.float32) for k, v in ins.items()}

    # host: trans + sort permutation (schedule only; guarded below)
    trans_x = C_DV * ins["ori_re"] + C_TN * ins["tn_re"]
    trans_y = C_DV * ins["ori_im"] + C_TN * ins["tn_im"]
    p0x = ins["pos_re"] + trans_x
    p0y = ins["pos_im"] + trans_y
    perm = np.argsort(p0x, kind="stable")
    ins_s = {k: v[perm] for k, v in ins.items()}
    p0x_s, p0y_s = p0x[perm], p0y[perm]

    # ---- host-side band guards (exact) ----
    xs = ins_s["pos_re"].astype(np.float64)
    premax = np.maximum.accumulate(xs)
    sufmin = np.minimum.accumulate(xs[::-1])[::-1]
    ok_f = bool(np.all(sufmin[BF:] - premax[:-BF] > (RO + RC) * 1.001))
    gap128 = float(np.min(p0x_s[BC:].astype(np.float64)
                          - p0x_s[:-BC].astype(np.float64)))

    if not ok_f:
        out_full = _reference_numpy(**ins)
        return out_full

    if _BUILT is None:
        _BUILT = _build_nc()
    nc = _BUILT

    in_maps = _prepare_inputs(ins_s, p0x_s, p0y_s)
    r = run_bass_kernel_spmd(nc, in_maps, core_ids=list(range(NCORES)),
                             trace=TRACE)
    LAST_EXEC_NS = r.exec_time_ns
    res = r.results

    # collision guard: max displacement vs sorted-gap bound
    dmax = 0.0
    for c in range(NCORES):
        g = res[c]["o_guard"].astype(np.float64)
        dmax = max(dmax, float(np.max(np.sqrt(g[:, :, 0] + g[:, :, 1]))))
    gap_wrap = float(p0x_s[3584].astype(np.float64) - p0x_s[511].astype(np.float64))
    if not (gap128 > 2.0 * RC * 1.001 + 2.0 * dmax + 1e-8
            and gap_wrap > 2.0 * RC * 1.001 + 2.0 * dmax + 1e-8):
        return _reference_numpy(**ins)

    out_s = np.stack([_asm(res, "o_pos"), _asm(res, "o_ori"),
                      _asm(res, "o_osum"), _asm(res, "o_left"),
                      _asm(res, "o_right")], axis=0)
    out = np.empty_like(out_s)
    out[:, perm, :] = out_s
    return out


# revision 31
# speedup vs baseline: 1.7514x; 1.0533x over previous
"""Trainium2 Bass kernel for the ActiveParticles nn.Module (N=4096, 8 cores).

Strategy
--------
Host sorts particles by post-translation x coordinate (a pure permutation —
the reference computation is permutation-equivariant). In sorted space every
interaction radius becomes a *rank band*:
  - collision radius 2*RC  -> band +-128 ranks   (30-iteration loop)
  - orientation radius RO+RC -> band +-384 ranks (one-time masks)
Each of the 8 cores owns 512 consecutive sorted rows and computes only a
[128 x band] tile strip per row-tile. The collision loop allgathers the
updated 512x2 positions every iteration. Band validity is *proved* per input:
host-side exact gap checks + a device-computed max-displacement guard; if any
check fails, a dense numpy fallback (exact reference semantics) is used.

Outputs are produced in sorted space and unpermuted on host.
"""

import os
import numpy as np

# ---------------------------------------------------------------- constants
PI = 3.1415927410125732
N = 4096
L = 1.28e-3
VEL = 5e-7
RC = 3.15e-6
RR = 8e-6
RO = 2.5e-5
DT_TRANS = 1.4e-14
DR = 0.0028
GAMMA = 25.0
DT = 0.2
COLL_ITERS = 30

NCORES = 8
ROWS = N // NCORES          # 512 rows per core
NT = ROWS // 128            # 4 row-tiles per core

BC = 128                    # collision band (ranks each side)
CW = 2 * BC + 128           # collision tile window = 384
CWIN = ROWS + 2 * BC        # per-core collision window = 768

BF = 384                    # forward (mask) band
FW = 2 * BF + 128           # forward tile window = 896
FWIN = ROWS + 2 * BF        # per-core forward window = 1280

PAD = 128                   # sentinel pad rows each side of the shared array
NPAD = N + 2 * PAD          # 4352
SENT = np.float32(1.0e9)

F32 = np.float32
C2 = F32((2.0 * RC) ** 2)          # collision threshold on squared dist
FAC = F32(1.05 * RC)               # (2.1*RC)*0.5
RR2 = F32(RR ** 2)
RO2 = F32((RO + RC) ** 2)
C_DV = F32(DT * VEL)
C_TN = F32(np.sqrt(0.5) * np.sqrt(2.0 * DT_TRANS) * np.sqrt(DT))
C_GGD = F32(DT * GAMMA * DR)
C_RN = F32(np.sqrt(2.0 * DR) * np.sqrt(DT))
HALF_PI = F32(PI / 2)

_BUILT = None
TRACE = False               # test.py sets True to collect exec_time_ns
LAST_EXEC_NS = None


# =====================================================================
# Device program
# =====================================================================
def _build_nc():
    import concourse.bass as bass
    import concourse.mybir as mybir
    from concourse import tile, bass_isa, library_config
    from concourse import bacc

    f32 = mybir.dt.float32
    A = mybir.AluOpType
    AF = mybir.ActivationFunctionType

    nc = bacc.Bacc("TRN2", target_bir_lowering=False, debug=False,
                   num_devices=NCORES)

    # ---------------- I/O ----------------
    t_own = nc.dram_tensor("own8", [128, NT, 8], f32, kind="ExternalInput")
    t_cwin = nc.dram_tensor("cwin", [6, CWIN], f32, kind="ExternalInput")
    t_fwin = nc.dram_tensor("fwin", [4, FWIN], f32, kind="ExternalInput")
    t_fpx = nc.dram_tensor("fpx", [1, N], f32, kind="ExternalInput")
    t_fpy = nc.dram_tensor("fpy", [1, N], f32, kind="ExternalInput")
    t_eye = nc.dram_tensor("eye128", [128, 128], f32, kind="ExternalInput")

    o_pos = nc.dram_tensor("o_pos", [128, NT, 2], f32, kind="ExternalOutput")
    o_ori = nc.dram_tensor("o_ori", [128, NT, 2], f32, kind="ExternalOutput")
    o_osum = nc.dram_tensor("o_osum", [128, NT, 2], f32, kind="ExternalOutput")
    o_left = nc.dram_tensor("o_left", [128, NT, 2], f32, kind="ExternalOutput")
    o_right = nc.dram_tensor("o_right", [128, NT, 2], f32, kind="ExternalOutput")
    o_guard = nc.dram_tensor("o_guard", [128, NT, 2], f32, kind="ExternalOutput")

    rg = [list(range(NCORES))]

    with tile.TileContext(nc) as tc:
        dve, act, gps, dma = nc.vector, nc.scalar, nc.gpsimd, nc.sync
        with tc.tile_pool(name="main", bufs=1) as mp, \
             tc.tile_pool(name="dram", bufs=2, space="DRAM") as dp:

            def T(shape, name, dt=f32):
                return mp.tile(shape, dt, name=name, tag=name, bufs=1)

            # persistent SBUF state
            own = T([128, NT, 8], "own")
            cwx = T([1, CWIN], "cwx")
            cwy = T([1, CWIN], "cwy")
            cwo1 = T([1, CWIN], "cwo1")
            cwo2 = T([1, CWIN], "cwo2")
            cwt1 = T([1, CWIN], "cwt1")
            cwt2 = T([1, CWIN], "cwt2")
            fwx = T([1, FWIN], "fwx")
            fwy = T([1, FWIN], "fwy")
            fwo1 = T([1, FWIN], "fwo1")
            fwo2 = T([1, FWIN], "fwo2")
            eye = T([128, 128], "eye")
            fpx = T([1, N], "fpx")
            fpy = T([1, N], "fpy")
            px = T([128, NT], "px")
            py = T([128, NT], "py")
            px0 = T([128, NT], "px0")
            py0 = T([128, NT], "py0")
            gx = T([128, NT], "gx")
            gy = T([128, NT], "gy")
            sumx = T([128, NT], "sumx")
            sumy = T([128, NT], "sumy")
            nrt_ = T([128, NT], "nrt")
            sxt = T([128, NT], "sxt")
            syt = T([128, NT], "syt")
            oxt = T([128, NT], "oxt")
            oyt = T([128, NT], "oyt")
            cbias = T([128, 1], "cbias")
            cbias5 = T([128, 1], "cbias5")
            npxn = T([128, NT], "npxn")
            npyn = T([128, NT], "npyn")

            dma.dma_start(own[:], t_own[:])
            dma.dma_start(cwx[:], t_cwin[0:1, :])
            dma.dma_start(cwy[:], t_cwin[1:2, :])
            dma.dma_start(cwo1[:], t_cwin[2:3, :])
            dma.dma_start(cwo2[:], t_cwin[3:4, :])
            dma.dma_start(cwt1[:], t_cwin[4:5, :])
            dma.dma_start(cwt2[:], t_cwin[5:6, :])
            dma.dma_start(fwx[:], t_fwin[0:1, :])
            dma.dma_start(fwy[:], t_fwin[1:2, :])
            dma.dma_start(fwo1[:], t_fwin[2:3, :])
            dma.dma_start(fwo2[:], t_fwin[3:4, :])
            dma.dma_start(eye[:], t_eye[:])
            dma.dma_start(fpx[:], t_fpx[:])
            dma.dma_start(fpy[:], t_fpy[:])
            dve.memset(cbias[:], float(HALF_PI))
            dve.memset(cbias5[:], -0.5)


            def ow(k):
                return own[:, :, k]

            # seed p0 = pos + trans (own rows)
            dve.scalar_tensor_tensor(px[:], ow(2), float(C_DV), ow(0), A.mult, A.add)
            dve.scalar_tensor_tensor(px[:], ow(6), float(C_TN), px[:], A.mult, A.add)
            dve.scalar_tensor_tensor(py[:], ow(3), float(C_DV), ow(1), A.mult, A.add)
            dve.scalar_tensor_tensor(py[:], ow(7), float(C_TN), py[:], A.mult, A.add)
            dve.tensor_copy(px0[:], px[:])
            dve.tensor_copy(py0[:], py[:])
            dve.memset(gx[:], 0.0)
            dve.memset(gy[:], 0.0)

            # seed window p0 = pos + trans
            dve.scalar_tensor_tensor(cwx[:], cwo1[:], float(C_DV), cwx[:],
                                     A.mult, A.add)
            dve.scalar_tensor_tensor(cwx[:], cwt1[:], float(C_TN), cwx[:],
                                     A.mult, A.add)
            dve.scalar_tensor_tensor(cwy[:], cwo2[:], float(C_DV), cwy[:],
                                     A.mult, A.add)
            dve.scalar_tensor_tensor(cwy[:], cwt2[:], float(C_TN), cwy[:],
                                     A.mult, A.add)

            # =====================================================
            # FORWARD one-time: masks + reductions (band BF)
            # =====================================================
            with tc.tile_pool(name="fsb", bufs=2) as fp:
                pxb = fp.tile([128, FWIN], f32, name="pxb", tag="pxb", bufs=1)
                pyb = fp.tile([128, FWIN], f32, name="pyb", tag="pyb", bufs=1)
                oreb = fp.tile([128, FWIN], f32, name="oreb", tag="oreb", bufs=1)
                oimb = fp.tile([128, FWIN], f32, name="oimb", tag="oimb", bufs=1)
                gps.partition_broadcast(pxb[:], fwx[:])
                gps.partition_broadcast(pyb[:], fwy[:])
                gps.partition_broadcast(oreb[:], fwo1[:])
                gps.partition_broadcast(oimb[:], fwo2[:])

                for t in range(NT):
                    w0 = 128 * t
                    pxi = own[:, t, 0:1]
                    pyi = own[:, t, 1:2]
                    orei = own[:, t, 2:3]
                    oimi = own[:, t, 3:4]
                    fdx = fp.tile([128, FW], f32, tag="fdx")
                    fdy = fp.tile([128, FW], f32, tag="fdy")
                    ft1 = fp.tile([128, FW], f32, tag="ft1")
                    ft2 = fp.tile([128, FW], f32, tag="ft2")
                    fsq = fp.tile([128, FW], f32, tag="fsq")
                    fg = fp.tile([128, FW], f32, tag="fg")
                    fm1 = fp.tile([128, FW], f32, tag="fm1")
                    fm2 = fp.tile([128, FW], f32, tag="fm2")
                    fwrr = fp.tile([128, FW], f32, tag="fwrr")
                    fwro = fp.tile([128, FW], f32, tag="fwro")

                    dve.tensor_scalar(fdx[:], pxb[:, w0:w0 + FW], pxi, None,
                                      A.subtract)
                    dve.tensor_scalar(fdy[:], pyb[:, w0:w0 + FW], pyi, None,
                                      A.subtract)
                    dve.tensor_tensor(fdx[:, BF:BF + 128], fdx[:, BF:BF + 128],
                                      eye[:], A.add)
                    act.activation(ft1[:], fdx[:], AF.Square)
                    act.activation(ft2[:], fdy[:], AF.Square)
                    dve.tensor_tensor(fsq[:], ft1[:], ft2[:], A.add)
                    dve.tensor_scalar(fg[:], oimb[:, w0:w0 + FW], oimi, None,
                                      A.mult)
                    dve.scalar_tensor_tensor(fg[:], oreb[:, w0:w0 + FW], orei,
                                             fg[:], A.mult, A.add)
                    dve.tensor_scalar(fm1[:], fsq[:], float(RR2), None, A.is_le)
                    dve.tensor_scalar(fm2[:], fg[:], 0.0, None, A.is_gt)
                    dve.tensor_tensor(fwrr[:], fm1[:], fm2[:], A.mult)
                    dve.tensor_scalar(fwro[:], fsq[:], float(RO2), None, A.is_le)
                    dve.tensor_tensor(fwro[:, BF:BF + 128],
                                      fwro[:, BF:BF + 128], eye[:], A.max)

                    dve.tensor_scalar(fwrr[:], fwrr[:], 1.0, None, A.mult,
                                      A.add, accum_out=nrt_[:, t:t + 1])
                    dve.scalar_tensor_tensor(ft1[:], fwrr[:], 1.0,
                                             pxb[:, w0:w0 + FW],
                                             A.mult, A.mult,
                                             accum_out=sxt[:, t:t + 1])
                    dve.scalar_tensor_tensor(ft2[:], fwrr[:], 1.0,
                                             pyb[:, w0:w0 + FW],
                                             A.mult, A.mult,
                                             accum_out=syt[:, t:t + 1])
                    dve.scalar_tensor_tensor(fdx[:], fwro[:], 1.0,
                                             oreb[:, w0:w0 + FW],
                                             A.mult, A.mult,
                                             accum_out=oxt[:, t:t + 1])
                    dve.scalar_tensor_tensor(fdy[:], fwro[:], 1.0,
                                             oimb[:, w0:w0 + FW],
                                             A.mult, A.mult,
                                             accum_out=oyt[:, t:t + 1])

            # ---------------- per-particle forward math [128, NT] --------
            def tt(name):
                return T([128, NT], name)

            c1x = T([1, 16], "c1x")
            c1y = T([1, 16], "c1y")
            cmsxb16 = T([128, 16], "cmsxb16")
            cmsyb16 = T([128, 16], "cmsyb16")
            dve.memset(c1x[:], 0.0)
            dve.memset(c1y[:], 0.0)
            dve.tensor_reduce(c1x[:, 0:1], fpx[:], mybir.AxisListType.X, A.add)
            dve.tensor_reduce(c1y[:, 0:1], fpy[:], mybir.AxisListType.X, A.add)
            dve.tensor_scalar(c1x[:], c1x[:], float(F32(1.0 / N)), None, A.mult)
            dve.tensor_scalar(c1y[:], c1y[:], float(F32(1.0 / N)), None, A.mult)
            gps.partition_broadcast(cmsxb16[:], c1x[:])
            gps.partition_broadcast(cmsyb16[:], c1y[:])
            cmsxb = cmsxb16[:, 0:1]
            cmsyb = cmsyb16[:, 0:1]

            nrx = tt("nrx")
            inr = tt("inr")
            snr = tt("snr")
            dve.tensor_scalar(nrx[:], nrt_[:], 1.0, None, A.max)
            dve.reciprocal(inr[:], nrx[:])
            dve.tensor_scalar(snr[:], nrt_[:], 0.0, None, A.is_gt)
            dxv = tt("dxv")
            dyv = tt("dyv")
            w1 = tt("w1")
            w2 = tt("w2")
            dve.tensor_tensor(w1[:], sxt[:], inr[:], A.mult)
            dve.tensor_tensor(w2[:], ow(0), snr[:], A.mult)
            dve.tensor_tensor(dxv[:], w2[:], w1[:], A.subtract)
            dve.tensor_tensor(w1[:], syt[:], inr[:], A.mult)
            dve.tensor_tensor(w2[:], ow(1), snr[:], A.mult)
            dve.tensor_tensor(dyv[:], w2[:], w1[:], A.subtract)

            psx = tt("psx")
            psy = tt("psy")
            dve.tensor_scalar(psx[:], ow(0), cmsxb, -1.0,
                              A.subtract, A.mult)
            dve.tensor_scalar(psy[:], ow(1), cmsyb, -1.0,
                              A.subtract, A.mult)

            cd = tt("cd")
            sd = tt("sd")
            act.activation(cd[:], ow(4), AF.Sin, bias=cbias[:])
            act.activation(sd[:], ow(4), AF.Sin)
            ta = tt("ta")
            tb = tt("tb")
            tc_ = tt("tc_")
            td = tt("td")
            dve.tensor_tensor(ta[:], psx[:], cd[:], A.mult)
            dve.tensor_tensor(tb[:], psy[:], sd[:], A.mult)
            dve.tensor_tensor(tc_[:], psy[:], cd[:], A.mult)
            dve.tensor_tensor(td[:], psx[:], sd[:], A.mult)
            lx = tt("lx")
            ly = tt("ly")
            rx = tt("rx")
            ry = tt("ry")
            dve.tensor_tensor(lx[:], ta[:], tb[:], A.subtract)
            dve.tensor_tensor(ly[:], tc_[:], td[:], A.add)
            dve.tensor_tensor(rx[:], ta[:], tb[:], A.add)
            dve.tensor_tensor(ry[:], tc_[:], td[:], A.subtract)

            dl = tt("dl")
            dr_ = tt("dr_")
            dve.tensor_tensor(w1[:], lx[:], oxt[:], A.mult)
            dve.tensor_tensor(w2[:], ly[:], oyt[:], A.mult)
            dve.tensor_tensor(dl[:], w1[:], w2[:], A.add)
            dve.tensor_tensor(w1[:], rx[:], oxt[:], A.mult)
            dve.tensor_tensor(w2[:], ry[:], oyt[:], A.mult)
            dve.tensor_tensor(dr_[:], w1[:], w2[:], A.add)
            mlr = tt("mlr")
            mlrc = tt("mlrc")
            dve.tensor_tensor(w1[:], dl[:], dr_[:], A.subtract)
            dve.tensor_scalar(mlr[:], w1[:], 0.0, None, A.is_ge)
            dve.tensor_scalar(mlrc[:], mlr[:], -1.0, 1.0, A.mult, A.add)
            bx = tt("bx")
            by = tt("by")
            dve.tensor_tensor(w1[:], mlr[:], lx[:], A.mult)
            dve.tensor_tensor(w2[:], mlrc[:], rx[:], A.mult)
            dve.tensor_tensor(bx[:], w1[:], w2[:], A.add)
            dve.tensor_tensor(w1[:], mlr[:], ly[:], A.mult)
            dve.tensor_tensor(w2[:], mlrc[:], ry[:], A.mult)
            dve.tensor_tensor(by[:], w1[:], w2[:], A.add)

            hr = tt("hr")
            dve.tensor_tensor(w1[:], dxv[:], dxv[:], A.mult)
            dve.tensor_tensor(w2[:], dyv[:], dyv[:], A.mult)
            dve.tensor_tensor(w1[:], w1[:], w2[:], A.add)
            dve.tensor_scalar(hr[:], w1[:], 0.0, None, A.is_gt)
            hrc = tt("hrc")
            dve.tensor_scalar(hrc[:], hr[:], -1.0, 1.0, A.mult, A.add)
            zx = tt("zx")
            zy = tt("zy")
            dve.tensor_tensor(w1[:], hr[:], dxv[:], A.mult)
            dve.tensor_tensor(w2[:], hrc[:], bx[:], A.mult)
            dve.tensor_tensor(zx[:], w1[:], w2[:], A.add)
            dve.tensor_tensor(w1[:], hr[:], dyv[:], A.mult)
            dve.tensor_tensor(w2[:], hrc[:], by[:], A.mult)
            dve.tensor_tensor(zy[:], w1[:], w2[:], A.add)
            wx = tt("wx")
            wy = tt("wy")
            dve.tensor_tensor(w1[:], zx[:], ow(2), A.mult)
            dve.tensor_tensor(w2[:], zy[:], ow(3), A.mult)
            dve.tensor_tensor(wx[:], w1[:], w2[:], A.add)
            dve.tensor_tensor(w1[:], zy[:], ow(2), A.mult)
            dve.tensor_tensor(w2[:], zx[:], ow(3), A.mult)
            dve.tensor_tensor(wy[:], w1[:], w2[:], A.subtract)
            # sin(att) = sin(angle(z1) - angle(ori)) = wy / |w|
            satt = tt("satt")
            act.activation(w1[:], wx[:], AF.Square)
            act.activation(w2[:], wy[:], AF.Square)
            dve.tensor_tensor(w1[:], w1[:], w2[:], A.add)
            dve.tensor_scalar(w1[:], w1[:], 1.0e-37, None, A.max)
            dve.reciprocal(w1[:], w1[:])
            act.activation(w1[:], w1[:], AF.Sqrt)
            dve.tensor_tensor(satt[:], wy[:], w1[:], A.mult)
            th = tt("th")
            dve.tensor_scalar(w1[:], satt[:], float(C_GGD), None, A.mult)
            dve.scalar_tensor_tensor(th[:], ow(5), float(C_RN), w1[:], A.mult, A.add)
            cth = tt("cth")
            sth = tt("sth")
            act.activation(cth[:], th[:], AF.Sin, bias=cbias[:])
            act.activation(sth[:], th[:], AF.Sin)
            nox = tt("nox")
            noy = tt("noy")
            dve.tensor_tensor(w1[:], ow(2), cth[:], A.mult)
            dve.tensor_tensor(w2[:], ow(3), sth[:], A.mult)
            dve.tensor_tensor(nox[:], w1[:], w2[:], A.subtract)
            dve.tensor_tensor(w1[:], ow(3), cth[:], A.mult)
            dve.tensor_tensor(w2[:], ow(2), sth[:], A.mult)
            dve.tensor_tensor(noy[:], w1[:], w2[:], A.add)

            dma.dma_start(o_ori[:, :, 0], nox[:])
            dma.dma_start(o_ori[:, :, 1], noy[:])
            dma.dma_start(o_osum[:, :, 0], oxt[:])
            dma.dma_start(o_osum[:, :, 1], oyt[:])
            dma.dma_start(o_left[:, :, 0], lx[:])
            dma.dma_start(o_left[:, :, 1], ly[:])
            dma.dma_start(o_right[:, :, 0], rx[:])
            dma.dma_start(o_right[:, :, 1], ry[:])

            # =====================================================
            # COLLISION LOOP (30 iterations, band BC)
            # =====================================================
            ones1 = T([1, 128], "ones1")
            dve.memset(ones1[:], 1.0)
            npx = T([128, NT], "npx")
            npy = T([128, NT], "npy")
            tmpd = T([128, NT], "tmpd")

            with tc.tile_pool(name="csb", bufs=3) as cp, \
                 tc.tile_pool(name="cps", bufs=1, space="PSUM") as cpp:
                fx = cp.tile([1, NPAD], f32, name="fx", tag="fx", bufs=1)
                fy = cp.tile([1, NPAD], f32, name="fy", tag="fy", bufs=1)
                dve.memset(fx[:], float(SENT))
                dve.memset(fy[:], float(SENT))
                prev_agout = None
                woff_h = []
                for it in range(COLL_ITERS):
                    if it > 0:
                        if not woff_h:
                            woff_h.append(dve.snap(
                                dve.partition_id() * ROWS, donate=False))
                        woff = woff_h[0]
                        agx = prev_agout[:].rearrange("(r two) w -> two r w",
                                                      two=2)
                        dma.dma_start(fx[0:1, PAD:PAD + N], agx[0])
                        dma.dma_start(fy[0:1, PAD:PAD + N], agx[1])
                        dve.tensor_copy(cwx[0:1, 0:CWIN],
                                        fx[0:1, bass.ds(woff, CWIN)])
                        dve.tensor_copy(cwy[0:1, 0:CWIN],
                                        fy[0:1, bass.ds(woff, CWIN)])
                    cpxb = cpp.tile([128, CWIN], f32, tag="bx", bufs=1)
                    cpyb = cpp.tile([128, CWIN], f32, tag="by", bufs=1)
                    nc.tensor.matmul(cpxb[:, 0:512], ones1[:], cwx[0:1, 0:512],
                                     start=True, stop=True)
                    nc.tensor.matmul(cpxb[:, 512:CWIN], ones1[:],
                                     cwx[0:1, 512:CWIN], start=True, stop=True)
                    nc.tensor.matmul(cpyb[:, 0:512], ones1[:], cwy[0:1, 0:512],
                                     start=True, stop=True)
                    nc.tensor.matmul(cpyb[:, 512:CWIN], ones1[:],
                                     cwy[0:1, 512:CWIN], start=True, stop=True)

                    cdx = cp.tile([128, NT, CW], f32, tag="cdx", bufs=1)
                    cdy = cp.tile([128, NT, CW], f32, tag="cdy", bufs=1)
                    ct1 = cp.tile([128, NT, CW], f32, tag="ct1", bufs=1)
                    ct2 = cp.tile([128, NT, CW], f32, tag="ct2", bufs=1)
                    csq = cp.tile([128, NT, CW], f32, tag="csq", bufs=1)
                    cfm = cp.tile([128, NT, CW], f32, tag="cfm", bufs=1)
                    cmx = cp.tile([128, NT, CW], f32, tag="cmx", bufs=1)
                    act.mul(npxn[:], px[:], -1.0)
                    act.mul(npyn[:], py[:], -1.0)
                    for t in range(NT):
                        w0 = 128 * t
                        act.activation(cdx[:, t, :], cpxb[:, w0:w0 + CW],
                                       AF.Identity, bias=npxn[:, t:t + 1])
                        dve.tensor_scalar(cdy[:, t, :], cpyb[:, w0:w0 + CW],
                                          py[:, t:t + 1], None, A.subtract)
                        dve.tensor_tensor(cdx[:, t, BC:BC + 128],
                                          cdx[:, t, BC:BC + 128], eye[:], A.add)
                    act.activation(ct1[:], cdx[:], AF.Square)
                    act.activation(ct2[:], cdy[:], AF.Square)
                    dve.tensor_tensor(csq[:], ct1[:], ct2[:], A.add)
                    # FAC/sqrt(csq) - 0.5 = rsqrt(csq/FAC^2) - 0.5
                    act.activation(ct2[:], csq[:], AF.Abs_reciprocal_sqrt,
                                   scale=float(F32(1.0 / (FAC * FAC))))
                    act.activation(ct2[:], ct2[:], AF.Identity, bias=cbias5[:])
                    dve.scalar_tensor_tensor(cfm[:], csq[:], float(C2), ct2[:],
                                             A.is_le, A.mult)
                    for t in range(NT):
                        dve.scalar_tensor_tensor(
                            cmx[:, t, :], cdx[:, t, :], 1.0, cfm[:, t, :],
                            A.mult, A.mult, accum_out=sumx[:, t:t + 1])
                        dve.scalar_tensor_tensor(
                            cmx[:, t, :], cdy[:, t, :], 1.0, cfm[:, t, :],
                            A.mult, A.mult, accum_out=sumy[:, t:t + 1])

                    dve.tensor_tensor(px[:], px[:], sumx[:], A.subtract)
                    dve.tensor_tensor(py[:], py[:], sumy[:], A.subtract)

                    if it < COLL_ITERS - 1:
                        agin = dp.tile([2, ROWS], f32, name="agin",
                                       tag="agin", bufs=3)
                        agout = dp.tile([2 * NCORES, ROWS], f32, name="agout",
                                        tag="agout", bufs=3,
                                        addr_space="Shared")
                        agr = agin[:].rearrange("c (t p) -> c p t", p=128)
                        dma.dma_start(agr[0], px[:])
                        dma.dma_start(agr[1], py[:])
                        gps.collective_compute(
                            "AllGather", A.bypass, replica_groups=rg,
                            ins=[agin[:].opt()],
                            outs=[agout[:].opt()])
                        prev_agout = agout

                    dve.tensor_tensor(tmpd[:], px[:], px0[:], A.subtract)
                    dve.tensor_tensor(tmpd[:], tmpd[:], tmpd[:], A.mult)
                    dve.tensor_tensor(gx[:], gx[:], tmpd[:], A.max)
                    dve.tensor_tensor(tmpd[:], py[:], py0[:], A.subtract)
                    dve.tensor_tensor(tmpd[:], tmpd[:], tmpd[:], A.mult)
                    dve.tensor_tensor(gy[:], gy[:], tmpd[:], A.max)

            dma.dma_start(o_pos[:, :, 0], px[:])
            dma.dma_start(o_pos[:, :, 1], py[:])
            dma.dma_start(o_guard[:, :, 0], gx[:])
            dma.dma_start(o_guard[:, :, 1], gy[:])

    nc.compile()
    return nc


# =====================================================================
# Host-side helpers
# =====================================================================
def _win(arr_s, c, band, fill):
    """arr_s padded window [c*ROWS - band, c*ROWS + ROWS + band)."""
    lo = c * ROWS - band
    hi = c * ROWS + ROWS + band
    out = np.full(hi - lo, fill, dtype=np.float32)
    a, b = max(lo, 0), min(hi, N)
    out[a - lo:b - lo] = arr_s[a:b]
    return out


def _prepare_inputs(ins_s, p0x_s, p0y_s):
    """Per-core input dicts from sorted arrays."""
    names = ["pos_re", "pos_im", "ori_re", "ori_im", "deltas", "rot_noise",
             "tn_re", "tn_im"]
    eye = np.eye(128, dtype=np.float32)
    fpx = ins_s["pos_re"][None, :].copy()
    fpy = ins_s["pos_im"][None, :].copy()
    in_maps = []
    for c in range(NCORES):
        sl = slice(c * ROWS, (c + 1) * ROWS)
        own = np.stack([ins_s[k][sl] for k in names], axis=-1)  # [512, 8]
        own = own.reshape(NT, 128, 8).transpose(1, 0, 2).copy()  # [128,4,8]

        cwin = np.stack([
            _win(ins_s["pos_re"], c, BC, SENT),
            _win(ins_s["pos_im"], c, BC, SENT),
            _win(ins_s["ori_re"], c, BC, 0.0),
            _win(ins_s["ori_im"], c, BC, 0.0),
            _win(ins_s["tn_re"], c, BC, 0.0),
            _win(ins_s["tn_im"], c, BC, 0.0),
        ])
        fwin = np.stack([
            _win(ins_s["pos_re"], c, BF, SENT),
            _win(ins_s["pos_im"], c, BF, SENT),
            _win(ins_s["ori_re"], c, BF, 0.0),
            _win(ins_s["ori_im"], c, BF, 0.0),
        ])
        in_maps.append({
            "own8": own, "cwin": cwin, "fwin": fwin, "fpx": fpx, "fpy": fpy,
            "eye128": eye,
        })
    return in_maps


def _reference_numpy(pos_re, pos_im, ori_re, ori_im, deltas, rot_noise,
                     tn_re, tn_im):
    """Dense fp32 fallback with exact reference semantics."""
    pos = (pos_re + 1j * pos_im).astype(np.complex64)
    ori = (ori_re + 1j * ori_im).astype(np.complex64)
    n = pos.shape[0]
    eye_b = np.eye(n, dtype=bool)
    eye_f = np.eye(n, dtype=np.float32)
    eye_c = eye_f.astype(np.complex64)

    dists = np.abs(pos[:, None] - pos[None, :] + eye_c).astype(np.float32)
    mask_rr = (dists <= RR) & ~eye_b
    mask_ro = (dists <= RO + RC) | eye_b

    def wrap(diff):
        diff = np.where(diff <= -PI, np.mod(diff, np.float32(PI)), diff)
        return diff - (diff >= PI).astype(np.float32) * np.float32(2.0 * PI)

    ang = np.angle(ori).astype(np.float32)
    abs_ad = np.abs(wrap(ang[:, None] - ang[None, :]))
    in_front = abs_ad < PI / 2

    Wrr = (mask_rr & in_front).astype(np.float32)
    Wro = mask_ro.astype(np.float32)

    def mvc(W, z):
        return (W @ z.real.astype(np.float32)
                + 1j * (W @ z.imag.astype(np.float32))).astype(np.complex64)

    n_r = Wrr.sum(axis=1)
    S = mvc(Wrr, pos) / np.maximum(n_r, 1.0) - pos * np.sign(n_r)
    d = -S
    cms = pos.sum() / np.float32(n)
    Ps = (cms - pos).astype(np.complex64)
    osum = mvc(Wro, ori)
    left = Ps * np.exp(1j * deltas).astype(np.complex64)
    right = Ps * np.exp(-1j * deltas).astype(np.complex64)

    def cossim(a, b):
        dot = a.real * b.real + a.imag * b.imag
        na = np.maximum(np.abs(a), 1e-14)
        nb = np.maximum(np.abs(b), 1e-14)
        return dot / (na * nb)

    best = np.where(cossim(left, osum) >= cossim(right, osum), left, right)
    has_rep = np.abs(d) > 0.0
    z1 = np.where(has_rep, d, best)
    att = wrap(np.angle(z1).astype(np.float32) - np.angle(ori).astype(np.float32))
    s2dr = np.float32(np.sqrt(2.0 * DR))
    sdt = np.float32(np.sqrt(DT))
    rot = np.exp(1j * (DT * GAMMA * DR * np.sin(att)
                       + rot_noise * s2dr * sdt)).astype(np.complex64)
    tnoise = ((tn_re + 1j * tn_im) * np.float32(np.sqrt(0.5))
              * np.float32(np.sqrt(2.0 * DT_TRANS))).astype(np.complex64)
    trans = (np.float32(DT * VEL) * ori + tnoise * sdt).astype(np.complex64)

    p = (pos + trans).astype(np.complex64)
    cont = True
    for _ in range(COLL_ITERS):
        diff = p[None, :] - p[:, None] + eye_c
        absd = np.abs(diff).astype(np.float32)
        coll = absd <= (2.0 * RC - eye_f)
        move = np.where(coll,
                        diff / np.where(coll, absd, 1.0)
                        * ((2.1 * RC - absd) * 0.5), 0.0)
        if cont:
            p = (p - move.sum(axis=1)).astype(np.complex64)
            cont = bool(coll.sum() > 0)
    new_pos = p
    new_ori = (ori * rot).astype(np.complex64)

    def c2r(z):
        return np.stack([z.real, z.imag], axis=-1).astype(np.float32)

    return np.stack([c2r(new_pos), c2r(new_ori), c2r(osum), c2r(left),
                     c2r(right)], axis=0)


def _asm(res, key):
    """[128, NT, 2] per-core outputs -> [N, 2] sorted order."""
    out = np.empty((N, 2), np.float32)
    for c in range(NCORES):
        a = res[c][key]  # [128, NT, 2]
        out[c * ROWS:(c + 1) * ROWS] = a.transpose(1, 0, 2).reshape(ROWS, 2)
    return out


def kernel(pos_re, pos_im, ori_re, ori_im, deltas, rot_noise, tn_re, tn_im):
    global _BUILT, LAST_EXEC_NS
    import sys
    sys.path.insert(0, "/opt/trn_rl_repo")
    from concourse.bass_utils import run_bass_kernel_spmd

    ins = dict(pos_re=pos_re, pos_im=pos_im, ori_re=ori_re, ori_im=ori_im,
               deltas=deltas, rot_noise=rot_noise, tn_re=tn_re, tn_im=tn_im)
    ins = {k: np.asarray(v, np.float32) for k, v in ins.items()}

    # host: trans + sort permutation (schedule only; guarded below)
    trans_x = C_DV * ins["ori_re"] + C_TN * ins["tn_re"]
    trans_y = C_DV * ins["ori_im"] + C_TN * ins["tn_im"]
    p0x = ins["pos_re"] + trans_x
    p0y = ins["pos_im"] + trans_y
    perm = np.argsort(p0x, kind="stable")
    ins_s = {k: v[perm] for k, v in ins.items()}
    p0x_s, p0y_s = p0x[perm], p0y[perm]

    # ---- host-side band guards (exact) ----
    xs = ins_s["pos_re"].astype(np.float64)
    premax = np.maximum.accumulate(xs)
    sufmin = np.minimum.accumulate(xs[::-1])[::-1]
    ok_f = bool(np.all(sufmin[BF:] - premax[:-BF] > (RO + RC) * 1.001))
    gap128 = float(np.min(p0x_s[BC:].astype(np.float64)
                          - p0x_s[:-BC].astype(np.float64)))

    if not ok_f:
        out_full = _reference_numpy(**ins)
        return out_full

    if _BUILT is None:
        _BUILT = _build_nc()
    nc = _BUILT

    in_maps = _prepare_inputs(ins_s, p0x_s, p0y_s)
    r = run_bass_kernel_spmd(nc, in_maps, core_ids=list(range(NCORES)),
                             trace=TRACE)
    LAST_EXEC_NS = r.exec_time_ns
    res = r.results

    # collision guard: max displacement vs sorted-gap bound
    dmax = 0.0
    for c in range(NCORES):
        g = res[c]["o_guard"].astype(np.float64)
        dmax = max(dmax, float(np.max(np.sqrt(g[:, :, 0] + g[:, :, 1]))))
    gap_wrap = float(p0x_s[3584].astype(np.float64) - p0x_s[511].astype(np.float64))
    if not (gap128 > 2.0 * RC * 1.001 + 2.0 * dmax + 1e-8
            and gap_wrap > 2.0 * RC * 1.001 + 2.0 * dmax + 1e-8):
        return _reference_numpy(**ins)

    out_s = np.stack([_asm(res, "o_pos"), _asm(res, "o_ori"),
                      _asm(res, "o_osum"), _asm(res, "o_left"),
                      _asm(res, "o_right")], axis=0)
    out = np.empty_like(out_s)
    out[:, perm, :] = out_s
    return out
